# revision 1
# baseline (speedup 1.0000x reference)
"""Block-causal (block=64) MHA + qkv/out projections on 8 NeuronCores.

Sharding: 8 cores = 2 batches x 4 head-groups (4 heads each).
Per core: qkv projection for its heads, block-causal attention for 4 heads
(processed as 2 head-pairs packed across the 128 partitions), partial output
projection over its 256 channels. Host sums the 4 partials per batch + bias.

On-chip layout is feature-major (transposed): scores are computed transposed
(S^T[k, q] = k . q) so no on-chip transposes are needed anywhere; softmax
denominators (sums over the key/partition axis) come from an all-ones matmul
on the PE, broadcast across 64 partitions. exp runs on ScalarE straight out
of PSUM. The diagonal 128-key tiles are split into two 64-key sub-blocks with
N-restricted matmuls, so block-causality costs no masking ops.
"""

import os

import numpy as np

import concourse.bass as bass
import concourse.tile as tile
from concourse import bacc
from concourse import mybir

B, N, C = 2, 2048, 1024
H, HD = 16, 64
HPC = 4  # heads per core
CSL = HPC * HD  # 256 channel slice per core
QKW = 2 * CSL  # 512: q then k output channels
NCORES = 8
QBLK = 512
NQB = N // QBLK  # 4
NT = N // 128  # 16 seq tiles of 128
SCALE = HD**-0.5
F32 = mybir.dt.float32
F32R = mybir.dt.float32r

USE_F32R = False


def _mm(ap):
    """Matmul operand view: compute in tf32-like float32r for full PE rate."""
    return ap.bitcast(F32R) if USE_F32R else ap


def build_nc():
    nc = bacc.Bacc("TRN2", target_bir_lowering=False, debug=False, num_devices=NCORES)

    xT_d = nc.dram_tensor("xT", [8, 128, N], F32, kind="ExternalInput")
    wqk_d = nc.dram_tensor("wqkT", [8, 128, QKW], F32, kind="ExternalInput")
    wv_d = nc.dram_tensor("wvT", [8, 128, CSL], F32, kind="ExternalInput")
    wp_d = nc.dram_tensor("wpT", [2, 128, C], F32, kind="ExternalInput")
    y_d = nc.dram_tensor("y", [N, C], F32, kind="ExternalOutput")

    with tile.TileContext(nc) as tc:
        with (
            tc.tile_pool(name="persist", bufs=1) as persist,
            tc.tile_pool(name="pt", bufs=2) as pt_pool,
            tc.tile_pool(name="rc", bufs=2) as rc_pool,
            tc.tile_pool(name="yout", bufs=3) as y_pool,
            tc.tile_pool(name="psmm", bufs=2, space="PSUM") as ps_mm,
            tc.tile_pool(name="pssc", bufs=1, space="PSUM") as ps_sc,
            tc.tile_pool(name="psacc", bufs=1, space="PSUM") as ps_acc,
        ):
            # ---- load inputs (one tile per DMA so consumers wait on few sems) ----
            xts = [persist.tile([128, N], F32, tag=f"xt{i}", name=f"xt{i}") for i in range(8)]
            wqks = [persist.tile([128, QKW], F32, tag=f"wqk{i}", name=f"wqk{i}") for i in range(8)]
            wvs = [persist.tile([128, CSL], F32, tag=f"wv{i}", name=f"wv{i}") for i in range(8)]
            wps = [persist.tile([128, C], F32, tag=f"wp{i}", name=f"wp{i}") for i in range(2)]
            for ct in range(8):
                nc.sync.dma_start(out=xts[ct], in_=xT_d[ct])
                nc.sync.dma_start(out=wqks[ct], in_=wqk_d[ct])
                nc.sync.dma_start(out=wvs[ct], in_=wv_d[ct])
            for pr in range(2):
                nc.sync.dma_start(out=wps[pr], in_=wp_d[pr])

            ones_t = persist.tile([128, 128], F32, tag="ones")
            nc.vector.memset(ones_t, 1.0)

            # ---- phase 1: q/k projection, transposed outputs ----
            # qkT tiles: 0 = q heads(0,1), 1 = q heads(2,3), 2 = k(0,1), 3 = k(3,4)
            # within a tile: partitions 0:64 = even head dims, 64:128 = odd head.
            qkT = [persist.tile([128, N], F32, tag=f"qk{t}", name=f"qk{t}") for t in range(4)]
            for dt_ in range(4):
                for nb in range(NQB):
                    ps = ps_mm.tile([128, QBLK], F32, tag="mm")
                    for ct in range(8):
                        nc.tensor.matmul(
                            ps,
                            lhsT=_mm(wqks[ct][:, dt_ * 128 : (dt_ + 1) * 128]),
                            rhs=_mm(xts[ct][:, nb * QBLK : (nb + 1) * QBLK]),
                            start=(ct == 0),
                            stop=(ct == 7),
                        )
                    nc.vector.tensor_copy(
                        out=qkT[dt_][:, nb * QBLK : (nb + 1) * QBLK], in_=ps
                    )

            # ---- phase 2: v projection, natural layout [n, 4*64] ----
            v_sb = [persist.tile([128, CSL], F32, tag=f"v{t}", name=f"v{t}") for t in range(NT)]
            for nt in range(NT):
                ps = ps_mm.tile([128, CSL], F32, tag="mm")
                for ct in range(8):
                    nc.tensor.matmul(
                        ps,
                        lhsT=_mm(xts[ct][:, nt * 128 : (nt + 1) * 128]),
                        rhs=_mm(wvs[ct]),
                        start=(ct == 0),
                        stop=(ct == 7),
                    )
                nc.vector.tensor_copy(out=v_sb[nt], in_=ps)

            # ---- phase 3+4: attention (per 512-query block), then out-proj ----
            PHASES = int(os.environ.get("KERNEL_PHASES", "3"))
            attnT = [persist.tile([128, N], F32, tag=f"at{p}", name=f"at{p}") for p in range(2)]
            if PHASES == 1:
                for p in range(2):
                    nc.vector.memset(attnT[p], 0.0)
            for qi in range(NQB if PHASES >= 2 else 0):
                for pair in range(2):
                    qt = qkT[pair]
                    kt_t = qkT[2 + pair]
                    qs = slice(qi * QBLK, (qi + 1) * QBLK)

                    # one PSUM bank per head per accumulator: the psum
                    # accumulation-group tracking cannot mix base-partition-0
                    # and base-partition-64 groups in one bank.
                    at_bA = ps_acc.tile([128, QBLK], F32, tag="atA", name="at_bA")
                    at_bB = ps_acc.tile([128, QBLK], F32, tag="atB", name="at_bB")
                    sm_bA = ps_acc.tile([128, QBLK], F32, tag="smA", name="sm_bA")
                    sm_bB = ps_acc.tile([128, QBLK], F32, tag="smB", name="sm_bB")

                    n_reg = 4 * qi
                    ATT_RECT = os.environ.get("ATT_RECT", "0") == "1"
                    diag_upto = int(os.environ.get("ATT_DIAG_UPTO", "8"))
                    if qi * 2 + pair >= diag_upto:
                        ATT_RECT = True
                    if ATT_RECT:
                        n_reg = 4 * qi + 4  # probe: no diagonal handling at all
                    # per partition-range (head) accumulation-group flags:
                    # the sim/HW psum group model tracks start/stop per
                    # partition range, so each head brackets its own group.
                    n_per_range = n_reg + (0 if ATT_RECT else 4)
                    at_A, at_B, sm_A, sm_B = [0], [0], [0], [0]

                    def fl(cnt, total=n_per_range):
                        i = cnt[0]
                        cnt[0] += 1
                        return dict(start=(i == 0), stop=(i == total - 1))

                    # fully-causal key tiles: whole [128k x 512q] rectangles
                    for kt in range(n_reg):
                        ks = slice(kt * 128, (kt + 1) * 128)
                        psA = ps_sc.tile([128, QBLK], F32, tag="sA")
                        psB = ps_sc.tile([128, QBLK], F32, tag="sB")
                        nc.tensor.matmul(
                            psA, lhsT=_mm(kt_t[0:64, ks]), rhs=_mm(qt[0:64, qs]),
                            start=True, stop=True,
                        )
                        nc.tensor.matmul(
                            psB, lhsT=_mm(kt_t[64:128, ks]), rhs=_mm(qt[64:128, qs]),
                            start=True, stop=True,
                        )
                        pA = pt_pool.tile([128, QBLK], F32, tag="pA")
                        pB = pt_pool.tile([128, QBLK], F32, tag="pB")
                        nc.scalar.activation(
                            out=pA, in_=psA, func=mybir.ActivationFunctionType.Exp,
                            scale=SCALE,
                        )
                        nc.scalar.activation(
                            out=pB, in_=psB, func=mybir.ActivationFunctionType.Exp,
                            scale=SCALE,
                        )
                        vA = v_sb[kt][:, pair * 128 : pair * 128 + 64]
                        vB = v_sb[kt][:, pair * 128 + 64 : pair * 128 + 128]
                        nc.tensor.matmul(
                            at_bA[0:64, :], lhsT=_mm(vA), rhs=_mm(pA), **fl(at_A)
                        )
                        nc.tensor.matmul(
                            at_bB[64:128, :], lhsT=_mm(vB), rhs=_mm(pB), **fl(at_B)
                        )
                        nc.tensor.matmul(
                            sm_bA[0:64, :], lhsT=_mm(ones_t[:, 0:64]), rhs=_mm(pA),
                            **fl(sm_A),
                        )
                        nc.tensor.matmul(
                            sm_bB[64:128, :], lhsT=_mm(ones_t[:, 64:128]), rhs=_mm(pB),
                            **fl(sm_B),
                        )

                    # diagonal key tiles: two 64-key sub-blocks, N-restricted
                    for j in ([] if ATT_RECT else range(4)):
                        kt = 4 * qi + j
                        q0 = 128 * j  # first allowed q offset for keys [0,64)
                        q1 = 128 * j + 64  # for keys [64,128)
                        if os.environ.get("ATT_DIAG_FULLN", "0") == "1":
                            q0 = q1 = 0  # probe: quadrant MMs, full N
                        k0 = slice(kt * 128, kt * 128 + 64)
                        k1 = slice(kt * 128 + 64, (kt + 1) * 128)
                        psA = ps_sc.tile([128, QBLK], F32, tag="sA")
                        psB = ps_sc.tile([128, QBLK], F32, tag="sB")
                        qsl0 = slice(qi * QBLK + q0, (qi + 1) * QBLK)
                        qsl1 = slice(qi * QBLK + q1, (qi + 1) * QBLK)
                        pA = pt_pool.tile([128, QBLK], F32, tag="pA")
                        pB = pt_pool.tile([128, QBLK], F32, tag="pB")
                        for ph, ps_s, p_s in ((0, psA, pA), (64, psB, pB)):
                            hd_sl = slice(ph, ph + 64)
                            # sub1 computes from q0 (not q1) so the bank is
                            # fully written and ONE exp covers both halves —
                            # two exps would read the bank while the second
                            # sub-MM still writes it (fatal PSUM collision).
                            nc.tensor.matmul(
                                ps_s[0:64, q0:QBLK], lhsT=_mm(kt_t[hd_sl, k0]),
                                rhs=_mm(qt[hd_sl, qsl0]), start=True, stop=True,
                            )
                            nc.tensor.matmul(
                                ps_s[64:128, q0:QBLK], lhsT=_mm(kt_t[hd_sl, k1]),
                                rhs=_mm(qt[hd_sl, qsl0]), start=True, stop=True,
                            )
                            nc.scalar.activation(
                                out=p_s[:, q0:QBLK], in_=ps_s[:, q0:QBLK],
                                func=mybir.ActivationFunctionType.Exp, scale=SCALE,
                            )
                            # zero the disallowed corner (keys k1 x queries
                            # [q0,q1)) so PV/sum can run as single K=128
                            # matmuls. Two row-split accumulating MMs would
                            # drain concurrently into the same PSUM cells —
                            # a fatal collision on hardware.
                            nc.gpsimd.memset(p_s[64:128, q0:q1], 0.0)
                        for ph, p_s, at_c, sm_c, at_b, sm_b in (
                            (0, pA, at_A, sm_A, at_bA, sm_bA),
                            (64, pB, at_B, sm_B, at_bB, sm_bB),
                        ):
                            vc = pair * 128 + ph  # head col offset: 0/64
                            nc.tensor.matmul(
                                at_b[ph : ph + 64, q0:QBLK],
                                lhsT=_mm(v_sb[kt][:, vc : vc + 64]),
                                rhs=_mm(p_s[:, q0:QBLK]), **fl(at_c),
                            )
                            nc.tensor.matmul(
                                sm_b[ph : ph + 64, q0:QBLK],
                                lhsT=_mm(ones_t[:, ph : ph + 64]),
                                rhs=_mm(p_s[:, q0:QBLK]), **fl(sm_c),
                            )

                    # normalize: attnT[:, qblock] = at * (1 / sm), per head half
                    recip = rc_pool.tile([128, QBLK], F32, tag="rc")
                    nc.vector.reciprocal(out=recip[0:64, :], in_=sm_bA[0:64, :])
                    nc.vector.reciprocal(out=recip[64:128, :], in_=sm_bB[64:128, :])
                    nc.vector.tensor_mul(
                        out=attnT[pair][0:64, qs], in0=at_bA[0:64, :], in1=recip[0:64, :]
                    )
                    nc.vector.tensor_mul(
                        out=attnT[pair][64:128, qs], in0=at_bB[64:128, :],
                        in1=recip[64:128, :],
                    )

                # output projection for this query block's 4 row tiles
                for nt in (range(4 * qi, 4 * qi + 4) if PHASES >= 3 else []):
                    ysb = y_pool.tile([128, C], F32, tag="y")
                    for cb in range(2):
                        psy = ps_mm.tile([128, QBLK], F32, tag="mm")
                        for pr in range(2):
                            nc.tensor.matmul(
                                psy,
                                lhsT=_mm(attnT[pr][:, nt * 128 : (nt + 1) * 128]),
                                rhs=_mm(wps[pr][:, cb * QBLK : (cb + 1) * QBLK]),
                                start=(pr == 0),
                                stop=(pr == 1),
                            )
                        nc.vector.tensor_copy(
                            out=ysb[:, cb * QBLK : (cb + 1) * QBLK], in_=psy
                        )
                    nc.sync.dma_start(out=y_d[nt * 128 : (nt + 1) * 128, :], in_=ysb)

            if PHASES < 3:
                for nt in range(NT):
                    ysb = y_pool.tile([128, C], F32, tag="y", name="ysb_fb")
                    for cb in range(2):
                        psy = ps_mm.tile([128, QBLK], F32, tag="mm", name="psy_fb")
                        for pr in range(2):
                            nc.tensor.matmul(
                                psy,
                                lhsT=_mm(attnT[pr][:, nt * 128 : (nt + 1) * 128]),
                                rhs=_mm(wps[pr][:, cb * QBLK : (cb + 1) * QBLK]),
                                start=(pr == 0),
                                stop=(pr == 1),
                            )
                        nc.vector.tensor_copy(
                            out=ysb[:, cb * QBLK : (cb + 1) * QBLK], in_=psy
                        )
                    nc.sync.dma_start(out=y_d[nt * 128 : (nt + 1) * 128, :], in_=ysb)

    return nc


def _shard_inputs(x, w_qkv, w_proj):
    x = np.ascontiguousarray(np.asarray(x, dtype=np.float32))
    w_qkv = np.asarray(w_qkv, dtype=np.float32)
    w_proj = np.asarray(w_proj, dtype=np.float32)
    xT = [np.ascontiguousarray(x[b].T).reshape(8, 128, N) for b in range(B)]
    in_maps = []
    for c in range(NCORES):
        b, g = divmod(c, 4)
        r0 = 64 * HPC * g  # 256 * g
        wq = w_qkv[r0 : r0 + CSL, :]
        wk = w_qkv[C + r0 : C + r0 + CSL, :]
        wvs = w_qkv[2 * C + r0 : 2 * C + r0 + CSL, :]
        wqkT = np.ascontiguousarray(np.concatenate([wq, wk], axis=0).T)
        wvT = np.ascontiguousarray(wvs.T)
        wpT = np.ascontiguousarray(w_proj[:, r0 : r0 + CSL].T)
        in_maps.append(
            {
                "xT": xT[b],
                "wqkT": wqkT.reshape(8, 128, QKW),
                "wvT": wvT.reshape(8, 128, CSL),
                "wpT": wpT.reshape(2, 128, C),
            }
        )
    return in_maps


def run(x, w_qkv, w_proj, b_proj, trace=False, **spmd_kwargs):
    from concourse.bass_utils import run_bass_kernel_spmd

    in_maps = _shard_inputs(x, w_qkv, w_proj)
    nc = build_nc()
    nc.finalize()
    res = run_bass_kernel_spmd(
        nc, in_maps, core_ids=list(range(NCORES)), trace=trace, **spmd_kwargs
    )
    y = np.zeros((B, N, C), np.float32)
    for c in range(NCORES):
        y[c // 4] += res.results[c]["y"]
    y += np.asarray(b_proj, dtype=np.float32)[None, None, :]
    return y, res


def kernel(x, w_qkv, w_proj, b_proj):
    y, _ = run(x, w_qkv, w_proj, b_proj, trace=False)
    return y



# revision 22
# speedup vs baseline: 2.1705x; 2.1705x over previous
"""Block-causal (block=64) MHA + qkv/out projections on 8 NeuronCores.

Sharding: 8 cores = 2 batches x 4 head-groups (4 heads each).
Per core: qkv projection for its heads, block-causal attention for 4 heads
(processed as 2 head-pairs packed across the 128 partitions), partial output
projection over its 256 channels. Host sums the 4 bf16 partials per batch
and adds the bias.

On-chip layout is feature-major (transposed): scores are computed transposed
(S^T[k, q] = k . q) so no on-chip transposes are needed anywhere. All matmuls
run in bf16 at the full 1-cycle/row PE rate with cheap weight loads (fp32
runs 4x slower; f32r loads stationaries ~4x slower and its matmuls cannot
write partition-64-based PSUM destinations). Inputs are cast f32->bf16 on
load (x on DVE, weights on gpsimd); PSUM accumulation stays fp32.

Softmax denominators come for free from the PV matmul: each V tile carries
an appended all-ones column (plus zero padding to a full 128-wide stationary,
which costs nothing since matmul time is per output row), so PV accumulator
row 64 is the running sum of exp. Denominator reciprocals (one row per head
half) are computed with the fast approximate DVE reciprocal and broadcast
across 64 partitions with a K=1 ones-matmul on the PE. The B half's
normalized rows are moved to attnT partitions 64:128 by SBUF->SBUF DMAs
(engine ops cannot shift partitions; DMA can).

The diagonal 128-key tiles use one N-restricted full-dst matmul per head
half; the disallowed sub-block corner of the probability tile is zeroed by a
gpsimd memset so PV runs as single wide matmuls.
"""

import numpy as np

import concourse.bass as bass
import concourse.tile as tile
from concourse import bacc
from concourse import mybir

B, N, C = 2, 2048, 1024
H, HD = 16, 64
HPC = 4  # heads per core
CSL = HPC * HD  # 256 channel slice per core
QKW = 2 * CSL  # 512: q then k output channels
NCORES = 8
QBLK = 512
NQB = N // QBLK  # 4
NT = N // 128  # 16 seq tiles of 128
SCALE = HD**-0.5
F32 = mybir.dt.float32
BF16 = mybir.dt.bfloat16
EXP = mybir.ActivationFunctionType.Exp


def build_nc():
    nc = bacc.Bacc("TRN2", target_bir_lowering=False, debug=False, num_devices=NCORES)

    xT_d = nc.dram_tensor("xT", [8, 128, N], F32, kind="ExternalInput")
    wqk_d = nc.dram_tensor("wqkT", [8, 128, QKW], F32, kind="ExternalInput")
    wv_d = nc.dram_tensor("wvT", [8, 128, CSL], F32, kind="ExternalInput")
    wp_d = nc.dram_tensor("wpT", [2, 128, C], F32, kind="ExternalInput")
    y_d = nc.dram_tensor("y", [N, C], BF16, kind="ExternalOutput")

    with tile.TileContext(nc) as tc:
        with (
            tc.tile_pool(name="persist", bufs=1) as persist,
            tc.tile_pool(name="stage", bufs=2) as stage,
            tc.tile_pool(name="pt", bufs=2) as pt_pool,
            tc.tile_pool(name="rc", bufs=2) as rc_pool,
            tc.tile_pool(name="yout", bufs=3) as y_pool,
            tc.tile_pool(name="psmm", bufs=2, space="PSUM") as ps_mm,
            tc.tile_pool(name="pssc", bufs=2, space="PSUM") as ps_sc,
            tc.tile_pool(name="psacc", bufs=1, space="PSUM") as ps_acc,
        ):
            # bf16 constants, built as f32 memset + DVE cast (direct bf16
            # memsets and PSUM-input custom-DVE ops are correctness
            # suspects on this compiler).
            ones_f = persist.tile([128, 64], F32, tag="onesf")
            zeros_f = persist.tile([128, 64], F32, tag="zerosf")
            nc.vector.memset(ones_f, 1.0)
            nc.vector.memset(zeros_f, 0.0)
            # ones row at partition 64 for the K=1 reciprocal-broadcast
            # matmuls (the reciprocal rows also sit at partition 64: matmul
            # needs equal lhsT/rhs bases, and PE out bases must be 0/32/64).
            onesel = persist.tile([128, 64], BF16, tag="sel")
            nc.vector.tensor_copy(out=onesel[64:65, :], in_=ones_f[64:65, :])
            zeros_b = persist.tile([128, 64], BF16, tag="zerosb")
            nc.vector.tensor_copy(out=zeros_b, in_=zeros_f)

            # ---- weight loads + on-chip f32 -> bf16 conversion ----
            wqk_bf = [persist.tile([128, QKW], BF16, tag=f"wqk{i}", name=f"wqk{i}") for i in range(8)]
            wv_bf = [persist.tile([128, CSL], BF16, tag=f"wv{i}", name=f"wv{i}") for i in range(8)]
            wp_bf = [persist.tile([128, C], BF16, tag=f"wp{i}", name=f"wp{i}") for i in range(2)]
            x_bf = [persist.tile([128, N], BF16, tag=f"xb{i}", name=f"xb{i}") for i in range(8)]

            wqk_st = []
            for ct in range(8):
                ws = stage.tile([128, QKW], F32, tag="ws", name=f"ws{ct}", bufs=2)
                nc.sync.dma_start(out=ws, in_=wqk_d[ct])
                wqk_st.append(ws)
            # x chunk loads: nb-major so phase 1 can start after 8 chunks
            x_st = {}
            for nb in range(NQB):
                for ct in range(8):
                    xs = stage.tile([128, QBLK], F32, tag="xs", name=f"xs{nb}_{ct}", bufs=4)
                    nc.sync.dma_start(out=xs, in_=xT_d[ct][:, nb * QBLK : (nb + 1) * QBLK])
                    x_st[(nb, ct)] = xs
            wv_st = []
            for ct in range(8):
                vs = stage.tile([128, CSL], F32, tag="vs", name=f"vs{ct}", bufs=2)
                nc.sync.dma_start(out=vs, in_=wv_d[ct])
                wv_st.append(vs)
            wp_st = []
            for pr in range(2):
                ws_ = stage.tile([128, C], F32, tag="wps", name=f"wps{pr}", bufs=2)
                nc.sync.dma_start(out=ws_, in_=wp_d[pr])
                wp_st.append(ws_)

            # weight casts on gpsimd (x casts go on DVE, interleaved below)
            for ct in range(8):
                nc.gpsimd.tensor_copy(out=wqk_bf[ct], in_=wqk_st[ct])
            for ct in range(8):
                nc.gpsimd.tensor_copy(out=wv_bf[ct], in_=wv_st[ct])
            for pr in range(2):
                nc.gpsimd.tensor_copy(out=wp_bf[pr], in_=wp_st[pr])

            def cast_x(nb):
                for ct in range(8):
                    nc.vector.tensor_copy(
                        out=x_bf[ct][:, nb * QBLK : (nb + 1) * QBLK], in_=x_st[(nb, ct)]
                    )

            # ---- phase 1: q/k projection, transposed bf16 outputs ----
            # qkT tiles: 0 = q heads(0,1), 1 = q heads(2,3), 2 = k(0,1), 3 = k(2,3)
            # within a tile: partitions 0:64 = even head dims, 64:128 = odd head.
            qkT = [persist.tile([128, N], BF16, tag=f"qk{t}", name=f"qk{t}") for t in range(4)]
            cast_x(0)
            cast_x(1)
            for nb in range(NQB):
                for dt_ in range(4):
                    ps = ps_mm.tile([128, QBLK], F32, tag="mm", name="ps_qk")
                    for ct in range(8):
                        nc.tensor.matmul(
                            ps,
                            lhsT=wqk_bf[ct][:, dt_ * 128 : (dt_ + 1) * 128],
                            rhs=x_bf[ct][:, nb * QBLK : (nb + 1) * QBLK],
                            start=(ct == 0),
                            stop=(ct == 7),
                        )
                    nc.vector.tensor_copy(
                        out=qkT[dt_][:, nb * QBLK : (nb + 1) * QBLK], in_=ps
                    )
                if nb + 2 < NQB:
                    cast_x(nb + 2)

            # ---- phase 2: v projection, with ones + zero-pad columns ----
            # vA/vB[nt] layout per pair p at cols 128p: [v(64) | ones | 0*63]
            # so the PV matmul writes a full 128-partition dst and its row 64
            # accumulates sum(exp); rows 65:128 accumulate zeros (free: cost
            # is per output row).
            vA = [persist.tile([128, 256], BF16, tag=f"vA{t}", name=f"vA{t}") for t in range(NT)]
            vB = [persist.tile([128, 256], BF16, tag=f"vB{t}", name=f"vB{t}") for t in range(NT)]
            for nt in range(NT):
                ps = ps_mm.tile([128, CSL], F32, tag="mm", name="ps_v")
                for ct in range(8):
                    nc.tensor.matmul(
                        ps,
                        lhsT=x_bf[ct][:, nt * 128 : (nt + 1) * 128],
                        rhs=wv_bf[ct],
                        start=(ct == 0),
                        stop=(ct == 7),
                    )
                ps3 = ps.rearrange("p (g c) -> p g c", c=128)  # [128, 2, 128]
                vA3 = vA[nt].rearrange("p (g c) -> p g c", c=128)
                vB3 = vB[nt].rearrange("p (g c) -> p g c", c=128)
                one3 = ones_f[:, 0:2].rearrange("p (g c) -> p g c", c=1)
                nc.vector.tensor_copy(out=vA3[:, :, 0:64], in_=ps3[:, :, 0:64])
                nc.vector.tensor_copy(out=vB3[:, :, 0:64], in_=ps3[:, :, 64:128])
                nc.vector.tensor_copy(out=vA3[:, :, 64:65], in_=one3)
                nc.vector.tensor_copy(out=vB3[:, :, 64:65], in_=one3)
                for g in range(2):
                    nc.vector.tensor_copy(out=vA3[:, g, 65:128], in_=zeros_f[:, 0:63])
                    nc.vector.tensor_copy(out=vB3[:, g, 65:128], in_=zeros_f[:, 0:63])

            # ---- phase 3+4: attention (per 512-query block), then out-proj ----
            attnT = [persist.tile([128, N], BF16, tag=f"at{p}", name=f"at{p}") for p in range(2)]
            for qi in range(NQB):
                for pair in range(2):
                    qt = qkT[pair]
                    kt_t = qkT[2 + pair]
                    qs = slice(qi * QBLK, (qi + 1) * QBLK)
                    vsl = slice(pair * 128, (pair + 1) * 128)

                    # one PSUM bank per head half, both at base partition 0,
                    # softmax denominator accumulating in row 64.
                    at_bA = ps_acc.tile([128, QBLK], F32, tag="atA", name="at_bA")
                    at_bB = ps_acc.tile([128, QBLK], F32, tag="atB", name="at_bB")

                    n_reg = 4 * qi
                    total = n_reg + 4
                    at_A, at_B = [0], [0]

                    def fl(cnt, t=total):
                        i = cnt[0]
                        cnt[0] += 1
                        return dict(start=(i == 0), stop=(i == t - 1))

                    # fully-causal key tiles: whole [128k x 512q] rectangles
                    for kt in range(n_reg):
                        ks = slice(kt * 128, (kt + 1) * 128)
                        psA = ps_sc.tile([128, QBLK], F32, tag="sA", name="psA")
                        psB = ps_sc.tile([128, QBLK], F32, tag="sB", name="psB")
                        nc.tensor.matmul(
                            psA, lhsT=kt_t[0:64, ks], rhs=qt[0:64, qs],
                            start=True, stop=True,
                        )
                        nc.tensor.matmul(
                            psB, lhsT=kt_t[64:128, ks], rhs=qt[64:128, qs],
                            start=True, stop=True,
                        )
                        pA = pt_pool.tile([128, QBLK], BF16, tag="pA", name="pA")
                        pB = pt_pool.tile([128, QBLK], BF16, tag="pB", name="pB")
                        nc.scalar.activation(out=pA, in_=psA, func=EXP, scale=SCALE)
                        nc.scalar.activation(out=pB, in_=psB, func=EXP, scale=SCALE)
                        nc.tensor.matmul(
                            at_bA, lhsT=vA[kt][:, vsl], rhs=pA, **fl(at_A)
                        )
                        nc.tensor.matmul(
                            at_bB, lhsT=vB[kt][:, vsl], rhs=pB, **fl(at_B)
                        )

                    # diagonal key tiles: one N-restricted full-dst matmul
                    # per half; keys 64:128 of the tile additionally need
                    # q >= q0 + 64, so that corner of p is zeroed before PV.
                    for j in range(4):
                        kt = 4 * qi + j
                        q0 = 128 * j
                        ks = slice(kt * 128, (kt + 1) * 128)
                        qsl0 = slice(qi * QBLK + q0, (qi + 1) * QBLK)
                        psA = ps_sc.tile([128, QBLK], F32, tag="sA", name="psAd")
                        psB = ps_sc.tile([128, QBLK], F32, tag="sB", name="psBd")
                        pA = pt_pool.tile([128, QBLK], BF16, tag="pA", name="pAd")
                        pB = pt_pool.tile([128, QBLK], BF16, tag="pB", name="pBd")
                        for ph, ps_s, p_s in ((0, psA, pA), (64, psB, pB)):
                            hd_sl = slice(ph, ph + 64)
                            nc.tensor.matmul(
                                ps_s[:, q0:QBLK], lhsT=kt_t[hd_sl, ks],
                                rhs=qt[hd_sl, qsl0], start=True, stop=True,
                            )
                            nc.scalar.activation(
                                out=p_s[:, q0:QBLK], in_=ps_s[:, q0:QBLK],
                                func=EXP, scale=SCALE,
                            )
                            nc.vector.tensor_copy(
                                out=p_s[64:128, q0 : q0 + 64], in_=zeros_b[64:128, :]
                            )
                        nc.tensor.matmul(
                            at_bA[:, q0:QBLK], lhsT=vA[kt][:, vsl],
                            rhs=pA[:, q0:QBLK], **fl(at_A)
                        )
                        nc.tensor.matmul(
                            at_bB[:, q0:QBLK], lhsT=vB[kt][:, vsl],
                            rhs=pB[:, q0:QBLK], **fl(at_B)
                        )

                    # normalize: fast approximate reciprocal of the two
                    # denominator rows, one bf16 cast, K=1 ones-matmul
                    # broadcast to 64 partitions (through SBUF: tensor ops
                    # read at most one PSUM input), multiply. B's rows are
                    # DMAd into attnT partitions 64:128 in 128-col chunks so
                    # out-proj row tiles unblock incrementally.
                    # reciprocal as exp(-ln d) on the Scalar engine: two
                    # table activations, no DVE and no custom ops (the
                    # custom-DVE approx reciprocal silently returns garbage
                    # on hardware, and nc.vector.reciprocal costs ~3.3us).
                    lg_t = rc_pool.tile([128, 2 * QBLK], F32, tag="lg", name="lg_t")
                    rc_b = rc_pool.tile([128, 2 * QBLK], BF16, tag="rcb", name="rc_b")
                    nc.scalar.activation(
                        out=lg_t[64:65, 0:QBLK], in_=at_bA[64:65, :],
                        func=mybir.ActivationFunctionType.Ln,
                    )
                    nc.scalar.activation(
                        out=lg_t[64:65, QBLK : 2 * QBLK], in_=at_bB[64:65, :],
                        func=mybir.ActivationFunctionType.Ln,
                    )
                    nc.scalar.activation(
                        out=rc_b[64:65, :], in_=lg_t[64:65, :], func=EXP, scale=-1.0
                    )
                    bcA = ps_sc.tile([128, QBLK], F32, tag="sA", name="bcA")
                    bcB = ps_sc.tile([128, QBLK], F32, tag="sB", name="bcB")
                    nc.tensor.matmul(
                        bcA[0:64, :], lhsT=onesel[64:65, :], rhs=rc_b[64:65, 0:QBLK],
                        start=True, stop=True,
                    )
                    nc.tensor.matmul(
                        bcB[0:64, :], lhsT=onesel[64:65, :],
                        rhs=rc_b[64:65, QBLK : 2 * QBLK], start=True, stop=True,
                    )
                    bsA = rc_pool.tile([64, QBLK], F32, tag="bsA", name="bsA")
                    bsB = rc_pool.tile([64, QBLK], F32, tag="bsB", name="bsB")
                    nc.vector.tensor_copy(out=bsA, in_=bcA[0:64, :])
                    nc.vector.tensor_copy(out=bsB, in_=bcB[0:64, :])
                    nc.vector.tensor_mul(
                        out=attnT[pair][0:64, qs], in0=at_bA[0:64, :], in1=bsA
                    )
                    tmpB = rc_pool.tile([64, QBLK], BF16, tag="tmpB", name="tmpB")
                    nc.vector.tensor_mul(out=tmpB, in0=at_bB[0:64, :], in1=bsB)
                    for ch in range(4):
                        nc.sync.dma_start(
                            out=attnT[pair][
                                64:128, qi * QBLK + ch * 128 : qi * QBLK + (ch + 1) * 128
                            ],
                            in_=tmpB[:, ch * 128 : (ch + 1) * 128],
                        )

                # output projection for this query block's 4 row tiles
                for nt in range(4 * qi, 4 * qi + 4):
                    ysb = y_pool.tile([128, C], BF16, tag="y", name="ysb")
                    for cb in range(2):
                        psy = ps_mm.tile([128, QBLK], F32, tag="mm", name="psy")
                        for pr in range(2):
                            nc.tensor.matmul(
                                psy,
                                lhsT=attnT[pr][:, nt * 128 : (nt + 1) * 128],
                                rhs=wp_bf[pr][:, cb * QBLK : (cb + 1) * QBLK],
                                start=(pr == 0),
                                stop=(pr == 1),
                            )
                        nc.vector.tensor_copy(
                            out=ysb[:, cb * QBLK : (cb + 1) * QBLK], in_=psy
                        )
                    nc.sync.dma_start(out=y_d[nt * 128 : (nt + 1) * 128, :], in_=ysb)

    return nc


def _shard_inputs(x, w_qkv, w_proj):
    x = np.ascontiguousarray(np.asarray(x, dtype=np.float32))
    w_qkv = np.asarray(w_qkv, dtype=np.float32)
    w_proj = np.asarray(w_proj, dtype=np.float32)
    xT = [np.ascontiguousarray(x[b].T).reshape(8, 128, N) for b in range(B)]
    in_maps = []
    for c in range(NCORES):
        b, g = divmod(c, 4)
        r0 = 64 * HPC * g  # 256 * g
        wq = w_qkv[r0 : r0 + CSL, :]
        wk = w_qkv[C + r0 : C + r0 + CSL, :]
        wvs = w_qkv[2 * C + r0 : 2 * C + r0 + CSL, :]
        wqkT = np.ascontiguousarray(np.concatenate([wq, wk], axis=0).T)
        wvT = np.ascontiguousarray(wvs.T)
        wpT = np.ascontiguousarray(w_proj[:, r0 : r0 + CSL].T)
        in_maps.append(
            {
                "xT": xT[b],
                "wqkT": wqkT.reshape(8, 128, QKW),
                "wvT": wvT.reshape(8, 128, CSL),
                "wpT": wpT.reshape(2, 128, C),
            }
        )
    return in_maps


def run(x, w_qkv, w_proj, b_proj, trace=False, **spmd_kwargs):
    from concourse.bass_utils import run_bass_kernel_spmd

    in_maps = _shard_inputs(x, w_qkv, w_proj)
    nc = build_nc()
    nc.finalize()
    res = run_bass_kernel_spmd(
        nc, in_maps, core_ids=list(range(NCORES)), trace=trace, **spmd_kwargs
    )
    y = np.zeros((B, N, C), np.float32)
    for c in range(NCORES):
        y[c // 4] += np.asarray(res.results[c]["y"], dtype=np.float32)
    y += np.asarray(b_proj, dtype=np.float32)[None, None, :]
    return y, res


def kernel(x, w_qkv, w_proj, b_proj):
    y, _ = run(x, w_qkv, w_proj, b_proj, trace=False)
    return y


# revision 23
# speedup vs baseline: 2.1955x; 1.0115x over previous
"""Block-causal (block=64) MHA + qkv/out projections on 8 NeuronCores.

Sharding: 8 cores = 2 batches x 4 head-groups (4 heads each).
Per core: qkv projection for its heads, block-causal attention for 4 heads
(processed as 2 head-pairs packed across the 128 partitions), partial output
projection over its 256 channels. Host sums the 4 bf16 partials per batch
and adds the bias.

On-chip layout is feature-major (transposed): scores are computed transposed
(S^T[k, q] = k . q) so no on-chip transposes are needed anywhere. All matmuls
run in bf16 at the full 1-cycle/row PE rate with cheap weight loads (fp32
runs 4x slower; f32r loads stationaries ~4x slower and its matmuls cannot
write partition-64-based PSUM destinations). Inputs are cast f32->bf16 on
load (x on DVE, weights on gpsimd); PSUM accumulation stays fp32.

Softmax denominators come for free from the PV matmul: each V tile carries
an appended all-ones column (plus zero padding to a full 128-wide stationary,
which costs nothing since matmul time is per output row), so PV accumulator
row 64 is the running sum of exp. Denominator reciprocals (one row per head
half) are computed with the fast approximate DVE reciprocal and broadcast
across 64 partitions with a K=1 ones-matmul on the PE. The B half's
normalized rows are moved to attnT partitions 64:128 by SBUF->SBUF DMAs
(engine ops cannot shift partitions; DMA can).

The diagonal 128-key tiles use one N-restricted full-dst matmul per head
half; the disallowed sub-block corner of the probability tile is zeroed by a
gpsimd memset so PV runs as single wide matmuls.
"""

import numpy as np

import concourse.bass as bass
import concourse.tile as tile
from concourse import bacc
from concourse import mybir

B, N, C = 2, 2048, 1024
H, HD = 16, 64
HPC = 4  # heads per core
CSL = HPC * HD  # 256 channel slice per core
QKW = 2 * CSL  # 512: q then k output channels
NCORES = 8
QBLK = 512
NQB = N // QBLK  # 4
NT = N // 128  # 16 seq tiles of 128
SCALE = HD**-0.5
F32 = mybir.dt.float32
BF16 = mybir.dt.bfloat16
EXP = mybir.ActivationFunctionType.Exp


def build_nc():
    nc = bacc.Bacc("TRN2", target_bir_lowering=False, debug=False, num_devices=NCORES)

    xT_d = nc.dram_tensor("xT", [8, 128, N], F32, kind="ExternalInput")
    wqk_d = nc.dram_tensor("wqkT", [8, 128, QKW], F32, kind="ExternalInput")
    wv_d = nc.dram_tensor("wvT", [8, 128, CSL], F32, kind="ExternalInput")
    wp_d = nc.dram_tensor("wpT", [2, 128, C], F32, kind="ExternalInput")
    y_d = nc.dram_tensor("y", [N, C], BF16, kind="ExternalOutput")

    with tile.TileContext(nc) as tc:
        with (
            tc.tile_pool(name="persist", bufs=1) as persist,
            tc.tile_pool(name="stage", bufs=2) as stage,
            tc.tile_pool(name="pt", bufs=2) as pt_pool,
            tc.tile_pool(name="rc", bufs=2) as rc_pool,
            tc.tile_pool(name="yout", bufs=3) as y_pool,
            tc.tile_pool(name="psmm", bufs=2, space="PSUM") as ps_mm,
            tc.tile_pool(name="pssc", bufs=2, space="PSUM") as ps_sc,
            tc.tile_pool(name="psacc", bufs=1, space="PSUM") as ps_acc,
        ):
            # bf16 constants, built as f32 memset + DVE cast (direct bf16
            # memsets and PSUM-input custom-DVE ops are correctness
            # suspects on this compiler).
            ones_f = persist.tile([128, 64], F32, tag="onesf")
            zeros_f = persist.tile([128, 64], F32, tag="zerosf")
            nc.vector.memset(ones_f, 1.0)
            nc.vector.memset(zeros_f, 0.0)
            # ones row at partition 64 for the K=1 reciprocal-broadcast
            # matmuls (the reciprocal rows also sit at partition 64: matmul
            # needs equal lhsT/rhs bases, and PE out bases must be 0/32/64).
            onesel = persist.tile([128, 64], BF16, tag="sel")
            nc.vector.tensor_copy(out=onesel[64:65, :], in_=ones_f[64:65, :])
            zeros_b = persist.tile([128, 64], BF16, tag="zerosb")
            nc.vector.tensor_copy(out=zeros_b, in_=zeros_f)

            # ---- weight loads + on-chip f32 -> bf16 conversion ----
            wqk_bf = [persist.tile([128, QKW], BF16, tag=f"wqk{i}", name=f"wqk{i}") for i in range(8)]
            wv_bf = [persist.tile([128, CSL], BF16, tag=f"wv{i}", name=f"wv{i}") for i in range(8)]
            wp_bf = [persist.tile([128, C], BF16, tag=f"wp{i}", name=f"wp{i}") for i in range(2)]
            x_bf = [persist.tile([128, N], BF16, tag=f"xb{i}", name=f"xb{i}") for i in range(8)]

            wqk_st = []
            for ct in range(8):
                ws = stage.tile([128, QKW], F32, tag="ws", name=f"ws{ct}", bufs=2)
                nc.sync.dma_start(out=ws, in_=wqk_d[ct])
                wqk_st.append(ws)
            # x chunk loads: nb-major so phase 1 can start after 8 chunks
            x_st = {}
            for nb in range(NQB):
                for ct in range(8):
                    xs = stage.tile([128, QBLK], F32, tag="xs", name=f"xs{nb}_{ct}", bufs=4)
                    nc.sync.dma_start(out=xs, in_=xT_d[ct][:, nb * QBLK : (nb + 1) * QBLK])
                    x_st[(nb, ct)] = xs
            wv_st = []
            for ct in range(8):
                vs = stage.tile([128, CSL], F32, tag="vs", name=f"vs{ct}", bufs=2)
                nc.sync.dma_start(out=vs, in_=wv_d[ct])
                wv_st.append(vs)
            wp_st = []
            for pr in range(2):
                ws_ = stage.tile([128, C], F32, tag="wps", name=f"wps{pr}", bufs=2)
                nc.sync.dma_start(out=ws_, in_=wp_d[pr])
                wp_st.append(ws_)

            # weight casts on gpsimd (x casts go on DVE, interleaved below)
            for ct in range(8):
                nc.gpsimd.tensor_copy(out=wqk_bf[ct], in_=wqk_st[ct])
            for ct in range(8):
                nc.gpsimd.tensor_copy(out=wv_bf[ct], in_=wv_st[ct])
            for pr in range(2):
                nc.gpsimd.tensor_copy(out=wp_bf[pr], in_=wp_st[pr])

            def cast_x(nb):
                for ct in range(8):
                    nc.vector.tensor_copy(
                        out=x_bf[ct][:, nb * QBLK : (nb + 1) * QBLK], in_=x_st[(nb, ct)]
                    )

            # ---- phase 1: q/k projection, transposed bf16 outputs ----
            # qkT tiles: 0 = q heads(0,1), 1 = q heads(2,3), 2 = k(0,1), 3 = k(2,3)
            # within a tile: partitions 0:64 = even head dims, 64:128 = odd head.
            qkT = [persist.tile([128, N], BF16, tag=f"qk{t}", name=f"qk{t}") for t in range(4)]
            cast_x(0)
            cast_x(1)
            for nb in range(NQB):
                for dt_ in range(4):
                    ps = ps_mm.tile([128, QBLK], F32, tag="mm", name="ps_qk")
                    for ct in range(8):
                        nc.tensor.matmul(
                            ps,
                            lhsT=wqk_bf[ct][:, dt_ * 128 : (dt_ + 1) * 128],
                            rhs=x_bf[ct][:, nb * QBLK : (nb + 1) * QBLK],
                            start=(ct == 0),
                            stop=(ct == 7),
                        )
                    nc.vector.tensor_copy(
                        out=qkT[dt_][:, nb * QBLK : (nb + 1) * QBLK], in_=ps
                    )
                if nb + 2 < NQB:
                    cast_x(nb + 2)

            # ---- phase 2: v projection, with ones + zero-pad columns ----
            # vA/vB[nt] layout per pair p at cols 128p: [v(64) | ones | 0*63]
            # so the PV matmul writes a full 128-partition dst and its row 64
            # accumulates sum(exp); rows 65:128 accumulate zeros (free: cost
            # is per output row).
            vA = [persist.tile([128, 256], BF16, tag=f"vA{t}", name=f"vA{t}") for t in range(NT)]
            vB = [persist.tile([128, 256], BF16, tag=f"vB{t}", name=f"vB{t}") for t in range(NT)]
            for nt in range(NT):
                ps = ps_mm.tile([128, CSL], F32, tag="mm", name="ps_v")
                for ct in range(8):
                    nc.tensor.matmul(
                        ps,
                        lhsT=x_bf[ct][:, nt * 128 : (nt + 1) * 128],
                        rhs=wv_bf[ct],
                        start=(ct == 0),
                        stop=(ct == 7),
                    )
                ps3 = ps.rearrange("p (g c) -> p g c", c=128)  # [128, 2, 128]
                vA3 = vA[nt].rearrange("p (g c) -> p g c", c=128)
                vB3 = vB[nt].rearrange("p (g c) -> p g c", c=128)
                one3 = ones_f[:, 0:2].rearrange("p (g c) -> p g c", c=1)
                nc.vector.tensor_copy(out=vA3[:, :, 0:64], in_=ps3[:, :, 0:64])
                nc.vector.tensor_copy(out=vB3[:, :, 0:64], in_=ps3[:, :, 64:128])
                nc.vector.tensor_copy(out=vA3[:, :, 64:65], in_=one3)
                nc.vector.tensor_copy(out=vB3[:, :, 64:65], in_=one3)
                for g in range(2):
                    nc.vector.tensor_copy(out=vA3[:, g, 65:128], in_=zeros_f[:, 0:63])
                    nc.vector.tensor_copy(out=vB3[:, g, 65:128], in_=zeros_f[:, 0:63])

            # ---- phase 3+4: attention (per 512-query block), then out-proj ----
            # Emission is software-pipelined so the PE never waits on the
            # exp/normalize chains: scores for step s+1 are emitted before
            # the PV of step s; a pair's normalization is deferred until
            # after the NEXT pair's first scores; the out-projection of a
            # query block is deferred into the next block's second pair.
            attnT = [persist.tile([128, N], BF16, tag=f"at{p}", name=f"at{p}") for p in range(2)]
            norm_q = []
            out_q = []

            def make_norm(pair, qs, at_bA, at_bB):
                def norm():
                    # reciprocal as exp(-ln d) on the Scalar engine: table
                    # activations only (the custom-DVE approx reciprocal
                    # silently returns garbage on hardware, and
                    # nc.vector.reciprocal costs ~3.3us a call). Broadcast
                    # across 64 partitions with a K=1 ones-matmul, through
                    # SBUF (tensor ops read at most one PSUM input).
                    lg_t = rc_pool.tile([128, 2 * QBLK], F32, tag="lg", name="lg_t")
                    rc_b = rc_pool.tile([128, 2 * QBLK], BF16, tag="rcb", name="rc_b")
                    nc.scalar.activation(
                        out=lg_t[64:65, 0:QBLK], in_=at_bA[64:65, :],
                        func=mybir.ActivationFunctionType.Ln,
                    )
                    nc.scalar.activation(
                        out=lg_t[64:65, QBLK : 2 * QBLK], in_=at_bB[64:65, :],
                        func=mybir.ActivationFunctionType.Ln,
                    )
                    nc.scalar.activation(
                        out=rc_b[64:65, :], in_=lg_t[64:65, :], func=EXP, scale=-1.0
                    )
                    bcA = ps_sc.tile([128, QBLK], F32, tag="sA", name="bcA")
                    bcB = ps_sc.tile([128, QBLK], F32, tag="sB", name="bcB")
                    nc.tensor.matmul(
                        bcA[0:64, :], lhsT=onesel[64:65, :],
                        rhs=rc_b[64:65, 0:QBLK], start=True, stop=True,
                    )
                    nc.tensor.matmul(
                        bcB[0:64, :], lhsT=onesel[64:65, :],
                        rhs=rc_b[64:65, QBLK : 2 * QBLK], start=True, stop=True,
                    )
                    bsA = rc_pool.tile([64, QBLK], F32, tag="bsA", name="bsA")
                    bsB = rc_pool.tile([64, QBLK], F32, tag="bsB", name="bsB")
                    nc.vector.tensor_copy(out=bsA, in_=bcA[0:64, :])
                    nc.vector.tensor_copy(out=bsB, in_=bcB[0:64, :])
                    nc.vector.tensor_mul(
                        out=attnT[pair][0:64, qs], in0=at_bA[0:64, :], in1=bsA
                    )
                    tmpB = rc_pool.tile([64, QBLK], BF16, tag="tmpB", name="tmpB")
                    nc.vector.tensor_mul(out=tmpB, in0=at_bB[0:64, :], in1=bsB)
                    # B's normalized rows move to partitions 64:128 (engine
                    # ops cannot shift partitions; SBUF->SBUF DMA can)
                    nc.sync.dma_start(out=attnT[pair][64:128, qs], in_=tmpB)
                return norm

            def make_outproj(qi):
                def op():
                    for nt in range(4 * qi, 4 * qi + 4):
                        ysb = y_pool.tile([128, C], BF16, tag="y", name="ysb")
                        for cb in range(2):
                            psy = ps_mm.tile([128, QBLK], F32, tag="mm", name="psy")
                            for pr in range(2):
                                nc.tensor.matmul(
                                    psy,
                                    lhsT=attnT[pr][:, nt * 128 : (nt + 1) * 128],
                                    rhs=wp_bf[pr][:, cb * QBLK : (cb + 1) * QBLK],
                                    start=(pr == 0),
                                    stop=(pr == 1),
                                )
                            nc.vector.tensor_copy(
                                out=ysb[:, cb * QBLK : (cb + 1) * QBLK], in_=psy
                            )
                        nc.sync.dma_start(
                            out=y_d[nt * 128 : (nt + 1) * 128, :], in_=ysb
                        )
                return op

            for qi in range(NQB):
                for pair in range(2):
                    qt = qkT[pair]
                    kt_t = qkT[2 + pair]
                    qs = slice(qi * QBLK, (qi + 1) * QBLK)
                    vsl = slice(pair * 128, (pair + 1) * 128)

                    # one PSUM bank per head half, both at base partition 0,
                    # softmax denominator accumulating in row 64.
                    at_bA = ps_acc.tile([128, QBLK], F32, tag="atA", name="at_bA")
                    at_bB = ps_acc.tile([128, QBLK], F32, tag="atB", name="at_bB")

                    n_reg = 4 * qi
                    total = n_reg + 4
                    at_A, at_B = [0], [0]

                    def fl(cnt, t=total):
                        i = cnt[0]
                        cnt[0] += 1
                        return dict(start=(i == 0), stop=(i == t - 1))

                    steps = [("rect", kt) for kt in range(n_reg)]
                    steps += [("diag", j) for j in range(4)]
                    st = {}

                    def emit_scores(i, qt=qt, kt_t=kt_t, qs=qs, qi=qi, steps=steps, st=st):
                        kind, idx = steps[i]
                        psA = ps_sc.tile([128, QBLK], F32, tag="sA", name="psA")
                        psB = ps_sc.tile([128, QBLK], F32, tag="sB", name="psB")
                        pA = pt_pool.tile([128, QBLK], BF16, tag="pA", name="pA")
                        pB = pt_pool.tile([128, QBLK], BF16, tag="pB", name="pB")
                        if kind == "rect":
                            ks = slice(idx * 128, (idx + 1) * 128)
                            nc.tensor.matmul(
                                psA, lhsT=kt_t[0:64, ks], rhs=qt[0:64, qs],
                                start=True, stop=True,
                            )
                            nc.tensor.matmul(
                                psB, lhsT=kt_t[64:128, ks], rhs=qt[64:128, qs],
                                start=True, stop=True,
                            )
                            nc.scalar.activation(out=pA, in_=psA, func=EXP, scale=SCALE)
                            nc.scalar.activation(out=pB, in_=psB, func=EXP, scale=SCALE)
                            st[i] = (idx, 0, pA, pB)
                        else:
                            # diagonal tile: one N-restricted full-dst MM per
                            # half; keys 64:128 additionally need q >= q0+64,
                            # so that corner of p is zeroed before PV.
                            kt = 4 * qi + idx
                            q0 = 128 * idx
                            ks = slice(kt * 128, (kt + 1) * 128)
                            qsl0 = slice(qi * QBLK + q0, (qi + 1) * QBLK)
                            for ph, ps_s, p_s in ((0, psA, pA), (64, psB, pB)):
                                hd_sl = slice(ph, ph + 64)
                                nc.tensor.matmul(
                                    ps_s[:, q0:QBLK], lhsT=kt_t[hd_sl, ks],
                                    rhs=qt[hd_sl, qsl0], start=True, stop=True,
                                )
                                nc.scalar.activation(
                                    out=p_s[:, q0:QBLK], in_=ps_s[:, q0:QBLK],
                                    func=EXP, scale=SCALE,
                                )
                                nc.vector.tensor_copy(
                                    out=p_s[64:128, q0 : q0 + 64],
                                    in_=zeros_b[64:128, :],
                                )
                            st[i] = (kt, q0, pA, pB)

                    def emit_pv(i, vsl=vsl, st=st, fl=fl, at_A=at_A, at_B=at_B,
                                at_bA=at_bA, at_bB=at_bB):
                        kt, q0, pA, pB = st.pop(i)
                        nc.tensor.matmul(
                            at_bA[:, q0:QBLK], lhsT=vA[kt][:, vsl],
                            rhs=pA[:, q0:QBLK], **fl(at_A)
                        )
                        nc.tensor.matmul(
                            at_bB[:, q0:QBLK], lhsT=vB[kt][:, vsl],
                            rhs=pB[:, q0:QBLK], **fl(at_B)
                        )

                    emit_scores(0)
                    if norm_q:
                        norm_q.pop(0)()
                    if pair == 1 and out_q:
                        out_q.pop(0)()
                    for i in range(len(steps)):
                        if i + 1 < len(steps):
                            emit_scores(i + 1)
                        emit_pv(i)
                    norm_q.append(make_norm(pair, qs, at_bA, at_bB))
                out_q.append(make_outproj(qi))
            while norm_q:
                norm_q.pop(0)()
            while out_q:
                out_q.pop(0)()

    return nc


def _shard_inputs(x, w_qkv, w_proj):
    x = np.ascontiguousarray(np.asarray(x, dtype=np.float32))
    w_qkv = np.asarray(w_qkv, dtype=np.float32)
    w_proj = np.asarray(w_proj, dtype=np.float32)
    xT = [np.ascontiguousarray(x[b].T).reshape(8, 128, N) for b in range(B)]
    in_maps = []
    for c in range(NCORES):
        b, g = divmod(c, 4)
        r0 = 64 * HPC * g  # 256 * g
        wq = w_qkv[r0 : r0 + CSL, :]
        wk = w_qkv[C + r0 : C + r0 + CSL, :]
        wvs = w_qkv[2 * C + r0 : 2 * C + r0 + CSL, :]
        wqkT = np.ascontiguousarray(np.concatenate([wq, wk], axis=0).T)
        wvT = np.ascontiguousarray(wvs.T)
        wpT = np.ascontiguousarray(w_proj[:, r0 : r0 + CSL].T)
        in_maps.append(
            {
                "xT": xT[b],
                "wqkT": wqkT.reshape(8, 128, QKW),
                "wvT": wvT.reshape(8, 128, CSL),
                "wpT": wpT.reshape(2, 128, C),
            }
        )
    return in_maps


def run(x, w_qkv, w_proj, b_proj, trace=False, **spmd_kwargs):
    from concourse.bass_utils import run_bass_kernel_spmd

    in_maps = _shard_inputs(x, w_qkv, w_proj)
    nc = build_nc()
    nc.finalize()
    res = run_bass_kernel_spmd(
        nc, in_maps, core_ids=list(range(NCORES)), trace=trace, **spmd_kwargs
    )
    y = np.zeros((B, N, C), np.float32)
    for c in range(NCORES):
        y[c // 4] += np.asarray(res.results[c]["y"], dtype=np.float32)
    y += np.asarray(b_proj, dtype=np.float32)[None, None, :]
    return y, res


def kernel(x, w_qkv, w_proj, b_proj):
    y, _ = run(x, w_qkv, w_proj, b_proj, trace=False)
    return y


# revision 25
# speedup vs baseline: 2.2875x; 1.0419x over previous
"""Block-causal (block=64) MHA + qkv/out projections on 8 NeuronCores.

Sharding: 8 cores = 2 batches x 4 head-groups (4 heads each).
Per core: qkv projection for its heads, block-causal attention for 4 heads
(processed as 2 head-pairs packed across the 128 partitions), partial output
projection over its 256 channels. Host sums the 4 bf16 partials per batch
and adds the bias.

On-chip layout is feature-major (transposed): scores are computed transposed
(S^T[k, q] = k . q) so no on-chip transposes are needed anywhere. All matmuls
run in bf16 at the full 1-cycle/row PE rate with cheap weight loads (fp32
runs 4x slower; f32r loads stationaries ~4x slower and its matmuls cannot
write partition-64-based PSUM destinations). Inputs are cast f32->bf16 on
load (x on DVE, weights on gpsimd); PSUM accumulation stays fp32.

Softmax denominators come for free from the PV matmul: each V tile carries
an appended all-ones column (plus zero padding to a full 128-wide stationary,
which costs nothing since matmul time is per output row), so PV accumulator
row 64 is the running sum of exp. Denominator reciprocals (one row per head
half) are computed with the fast approximate DVE reciprocal and broadcast
across 64 partitions with a K=1 ones-matmul on the PE. The B half's
normalized rows are moved to attnT partitions 64:128 by SBUF->SBUF DMAs
(engine ops cannot shift partitions; DMA can).

The diagonal 128-key tiles use one N-restricted full-dst matmul per head
half; the disallowed sub-block corner of the probability tile is zeroed by a
gpsimd memset so PV runs as single wide matmuls.
"""

import numpy as np

import concourse.bass as bass
import concourse.tile as tile
from concourse import bacc
from concourse import mybir

B, N, C = 2, 2048, 1024
H, HD = 16, 64
HPC = 4  # heads per core
CSL = HPC * HD  # 256 channel slice per core
QKW = 2 * CSL  # 512: q then k output channels
NCORES = 8
QBLK = 512
NQB = N // QBLK  # 4
NT = N // 128  # 16 seq tiles of 128
SCALE = HD**-0.5
F32 = mybir.dt.float32
BF16 = mybir.dt.bfloat16
EXP = mybir.ActivationFunctionType.Exp


def build_nc():
    nc = bacc.Bacc("TRN2", target_bir_lowering=False, debug=False, num_devices=NCORES)

    xT_d = nc.dram_tensor("xT", [8, 128, N], F32, kind="ExternalInput")
    wqk_d = nc.dram_tensor("wqkT", [8, 128, QKW], F32, kind="ExternalInput")
    wv_d = nc.dram_tensor("wvT", [8, 128, CSL], F32, kind="ExternalInput")
    wp_d = nc.dram_tensor("wpT", [2, 128, C], F32, kind="ExternalInput")
    y_d = nc.dram_tensor("y", [N, C], BF16, kind="ExternalOutput")

    with tile.TileContext(nc) as tc:
        with (
            tc.tile_pool(name="persist", bufs=1) as persist,
            tc.tile_pool(name="stage", bufs=2) as stage,
            tc.tile_pool(name="pt", bufs=2) as pt_pool,
            tc.tile_pool(name="rc", bufs=2) as rc_pool,
            tc.tile_pool(name="yout", bufs=3) as y_pool,
            tc.tile_pool(name="psmm", bufs=2, space="PSUM") as ps_mm,
            tc.tile_pool(name="pssc", bufs=1, space="PSUM") as ps_sc,
            tc.tile_pool(name="psacc", bufs=2, space="PSUM") as ps_acc,
        ):
            # bf16 constants, built as f32 memset + DVE cast (direct bf16
            # memsets and PSUM-input custom-DVE ops are correctness
            # suspects on this compiler).
            ones_f = persist.tile([128, 64], F32, tag="onesf")
            zeros_f = persist.tile([128, 64], F32, tag="zerosf")
            nc.vector.memset(ones_f, 1.0)
            nc.vector.memset(zeros_f, 0.0)
            # ones row at partition 64 for the K=1 reciprocal-broadcast
            # matmuls (the reciprocal rows also sit at partition 64: matmul
            # needs equal lhsT/rhs bases, and PE out bases must be 0/32/64).
            onesel = persist.tile([128, 64], BF16, tag="sel")
            nc.vector.tensor_copy(out=onesel[64:65, :], in_=ones_f[64:65, :])
            zeros_b = persist.tile([128, 64], BF16, tag="zerosb")
            nc.vector.tensor_copy(out=zeros_b, in_=zeros_f)

            # ---- weight loads + on-chip f32 -> bf16 conversion ----
            wqk_bf = [persist.tile([128, QKW], BF16, tag=f"wqk{i}", name=f"wqk{i}") for i in range(8)]
            wv_bf = [persist.tile([128, CSL], BF16, tag=f"wv{i}", name=f"wv{i}") for i in range(8)]
            wp_bf = [persist.tile([128, C], BF16, tag=f"wp{i}", name=f"wp{i}") for i in range(2)]
            x_bf = [persist.tile([128, N], BF16, tag=f"xb{i}", name=f"xb{i}") for i in range(8)]

            wqk_st = []
            for ct in range(8):
                ws = stage.tile([128, QKW], F32, tag="ws", name=f"ws{ct}", bufs=2)
                nc.sync.dma_start(out=ws, in_=wqk_d[ct])
                wqk_st.append(ws)
            # x chunk loads: nb-major so phase 1 can start after 8 chunks
            x_st = {}
            for nb in range(NQB):
                for ct in range(8):
                    xs = stage.tile([128, QBLK], F32, tag="xs", name=f"xs{nb}_{ct}", bufs=4)
                    nc.sync.dma_start(out=xs, in_=xT_d[ct][:, nb * QBLK : (nb + 1) * QBLK])
                    x_st[(nb, ct)] = xs
            wv_st = []
            for ct in range(8):
                vs = stage.tile([128, CSL], F32, tag="vs", name=f"vs{ct}", bufs=2)
                nc.sync.dma_start(out=vs, in_=wv_d[ct])
                wv_st.append(vs)
            wp_st = []
            for pr in range(2):
                ws_ = stage.tile([128, C], F32, tag="wps", name=f"wps{pr}", bufs=2)
                nc.sync.dma_start(out=ws_, in_=wp_d[pr])
                wp_st.append(ws_)

            # weight casts on gpsimd (x casts go on DVE, interleaved below)
            for ct in range(8):
                nc.gpsimd.tensor_copy(out=wqk_bf[ct], in_=wqk_st[ct])
            for ct in range(8):
                nc.gpsimd.tensor_copy(out=wv_bf[ct], in_=wv_st[ct])
            for pr in range(2):
                nc.gpsimd.tensor_copy(out=wp_bf[pr], in_=wp_st[pr])

            def cast_x(nb):
                for ct in range(8):
                    nc.vector.tensor_copy(
                        out=x_bf[ct][:, nb * QBLK : (nb + 1) * QBLK], in_=x_st[(nb, ct)]
                    )

            # ---- phase 1: q/k projection, transposed bf16 outputs ----
            # qkT tiles: 0 = q heads(0,1), 1 = q heads(2,3), 2 = k(0,1), 3 = k(2,3)
            # within a tile: partitions 0:64 = even head dims, 64:128 = odd head.
            qkT = [persist.tile([128, N], BF16, tag=f"qk{t}", name=f"qk{t}") for t in range(4)]
            cast_x(0)
            cast_x(1)
            for nb in range(NQB):
                for dt_ in range(4):
                    ps = ps_mm.tile([128, QBLK], F32, tag="mm", name="ps_qk")
                    for ct in range(8):
                        nc.tensor.matmul(
                            ps,
                            lhsT=wqk_bf[ct][:, dt_ * 128 : (dt_ + 1) * 128],
                            rhs=x_bf[ct][:, nb * QBLK : (nb + 1) * QBLK],
                            start=(ct == 0),
                            stop=(ct == 7),
                        )
                    nc.vector.tensor_copy(
                        out=qkT[dt_][:, nb * QBLK : (nb + 1) * QBLK], in_=ps
                    )
                if nb + 2 < NQB:
                    cast_x(nb + 2)

            # ---- phase 2: v projection, with ones + zero-pad columns ----
            # vA/vB[nt] layout per pair p at cols 128p: [v(64) | ones | 0*63]
            # so the PV matmul writes a full 128-partition dst and its row 64
            # accumulates sum(exp); rows 65:128 accumulate zeros (free: cost
            # is per output row).
            vA = [persist.tile([128, 256], BF16, tag=f"vA{t}", name=f"vA{t}") for t in range(NT)]
            vB = [persist.tile([128, 256], BF16, tag=f"vB{t}", name=f"vB{t}") for t in range(NT)]
            for nt in range(NT):
                ps = ps_mm.tile([128, CSL], F32, tag="mm", name="ps_v")
                for ct in range(8):
                    nc.tensor.matmul(
                        ps,
                        lhsT=x_bf[ct][:, nt * 128 : (nt + 1) * 128],
                        rhs=wv_bf[ct],
                        start=(ct == 0),
                        stop=(ct == 7),
                    )
                ps3 = ps.rearrange("p (g c) -> p g c", c=128)  # [128, 2, 128]
                vA3 = vA[nt].rearrange("p (g c) -> p g c", c=128)
                vB3 = vB[nt].rearrange("p (g c) -> p g c", c=128)
                one3 = ones_f[:, 0:2].rearrange("p (g c) -> p g c", c=1)
                nc.vector.tensor_copy(out=vA3[:, :, 0:64], in_=ps3[:, :, 0:64])
                nc.vector.tensor_copy(out=vB3[:, :, 0:64], in_=ps3[:, :, 64:128])
                nc.vector.tensor_copy(out=vA3[:, :, 64:65], in_=one3)
                nc.vector.tensor_copy(out=vB3[:, :, 64:65], in_=one3)
                for g in range(2):
                    nc.vector.tensor_copy(out=vA3[:, g, 65:128], in_=zeros_f[:, 0:63])
                    nc.vector.tensor_copy(out=vB3[:, g, 65:128], in_=zeros_f[:, 0:63])

            # ---- phase 3+4: attention (per 512-query block), then out-proj ----
            # Emission is software-pipelined so the PE never waits on the
            # exp/normalize chains: scores for step s+1 are emitted before
            # the PV of step s; a pair's normalization is deferred until
            # after the NEXT pair's first scores; the out-projection of a
            # query block is deferred into the next block's second pair.
            attnT = [persist.tile([128, N], BF16, tag=f"at{p}", name=f"at{p}") for p in range(2)]
            norm_q = []
            out_q = []

            def make_norm(pair, qs, at_bA, at_bB):
                def norm():
                    # reciprocal via the BITWISE_NOT seed + 2 Newton-Raphson
                    # steps, spelled out in STANDARD DVE ops (the fused
                    # custom-DVE op silently returns garbage on hardware;
                    # ACT Ln/Exp thrashes activation-table loads;
                    # nc.vector.reciprocal costs ~3.3us a call).
                    # ~bits(d) as f32 times c0 is a ~6% seed for 1/d.
                    C0, C1, C2 = -0.23549792, 2.0017324, 2.0
                    I32 = mybir.dt.int32
                    XOR = mybir.AluOpType.bitwise_xor
                    MUL = mybir.AluOpType.mult
                    ADD = mybir.AluOpType.add
                    w1 = rc_pool.tile([128, 2 * QBLK], F32, tag="w1", name="w1")
                    w2 = rc_pool.tile([128, 2 * QBLK], F32, tag="w2", name="w2")
                    w3 = rc_pool.tile([128, 2 * QBLK], F32, tag="w3", name="w3")
                    rc_b = rc_pool.tile([128, 2 * QBLK], BF16, tag="rcb", name="rc_b")
                    r = slice(64, 65)
                    halves = (
                        (at_bA[r, :], slice(0, QBLK)),
                        (at_bB[r, :], slice(QBLK, 2 * QBLK)),
                    )
                    for d, cs in halves:
                        nc.vector.tensor_scalar(
                            out=w1[r, cs].bitcast(I32), in0=d.bitcast(I32),
                            scalar1=-1, scalar2=None, op0=XOR,
                        )
                    nc.vector.tensor_scalar_mul(w2[r, :], w1[r, :], C0)  # y0
                    for d, cs in halves:
                        nc.vector.tensor_mul(out=w1[r, cs], in0=d, in1=w2[r, cs])
                    nc.vector.tensor_scalar(
                        out=w3[r, :], in0=w1[r, :], scalar1=-1.0, scalar2=C1,
                        op0=MUL, op1=ADD,
                    )
                    nc.vector.tensor_mul(out=w1[r, :], in0=w2[r, :], in1=w3[r, :])
                    for d, cs in halves:  # y1 in w1
                        nc.vector.tensor_mul(out=w2[r, cs], in0=d, in1=w1[r, cs])
                    nc.vector.tensor_scalar(
                        out=w3[r, :], in0=w2[r, :], scalar1=-1.0, scalar2=C2,
                        op0=MUL, op1=ADD,
                    )
                    nc.vector.tensor_mul(out=rc_b[r, :], in0=w1[r, :], in1=w3[r, :])
                    # broadcast across 64 partitions with K=1 ones-matmuls
                    # (PSUM from the mm ring), staged through SBUF (tensor
                    # ops read at most one PSUM input).
                    bcA = ps_mm.tile([128, QBLK], F32, tag="mm", name="bcA")
                    bcB = ps_mm.tile([128, QBLK], F32, tag="mm", name="bcB")
                    nc.tensor.matmul(
                        bcA[0:64, :], lhsT=onesel[64:65, :],
                        rhs=rc_b[64:65, 0:QBLK], start=True, stop=True,
                    )
                    nc.tensor.matmul(
                        bcB[0:64, :], lhsT=onesel[64:65, :],
                        rhs=rc_b[64:65, QBLK : 2 * QBLK], start=True, stop=True,
                    )
                    bsA = rc_pool.tile([64, QBLK], F32, tag="bsA", name="bsA")
                    bsB = rc_pool.tile([64, QBLK], F32, tag="bsB", name="bsB")
                    nc.vector.tensor_copy(out=bsA, in_=bcA[0:64, :])
                    nc.vector.tensor_copy(out=bsB, in_=bcB[0:64, :])
                    nc.vector.tensor_mul(
                        out=attnT[pair][0:64, qs], in0=at_bA[0:64, :], in1=bsA
                    )
                    tmpB = rc_pool.tile([64, QBLK], BF16, tag="tmpB", name="tmpB")
                    nc.vector.tensor_mul(out=tmpB, in0=at_bB[0:64, :], in1=bsB)
                    # B's normalized rows move to partitions 64:128 (engine
                    # ops cannot shift partitions; SBUF->SBUF DMA can)
                    nc.sync.dma_start(out=attnT[pair][64:128, qs], in_=tmpB)
                return norm

            def make_outproj(qi):
                def op():
                    for nt in range(4 * qi, 4 * qi + 4):
                        ysb = y_pool.tile([128, C], BF16, tag="y", name="ysb")
                        for cb in range(2):
                            psy = ps_mm.tile([128, QBLK], F32, tag="mm", name="psy")
                            for pr in range(2):
                                nc.tensor.matmul(
                                    psy,
                                    lhsT=attnT[pr][:, nt * 128 : (nt + 1) * 128],
                                    rhs=wp_bf[pr][:, cb * QBLK : (cb + 1) * QBLK],
                                    start=(pr == 0),
                                    stop=(pr == 1),
                                )
                            nc.vector.tensor_copy(
                                out=ysb[:, cb * QBLK : (cb + 1) * QBLK], in_=psy
                            )
                        nc.sync.dma_start(
                            out=y_d[nt * 128 : (nt + 1) * 128, :], in_=ysb
                        )
                return op

            for qi in range(NQB):
                for pair in range(2):
                    qt = qkT[pair]
                    kt_t = qkT[2 + pair]
                    qs = slice(qi * QBLK, (qi + 1) * QBLK)
                    vsl = slice(pair * 128, (pair + 1) * 128)

                    # one PSUM bank per head half, both at base partition 0,
                    # softmax denominator accumulating in row 64.
                    at_bA = ps_acc.tile([128, QBLK], F32, tag="atA", name="at_bA")
                    at_bB = ps_acc.tile([128, QBLK], F32, tag="atB", name="at_bB")

                    n_reg = 4 * qi
                    total = n_reg + 4
                    at_A, at_B = [0], [0]

                    def fl(cnt, t=total):
                        i = cnt[0]
                        cnt[0] += 1
                        return dict(start=(i == 0), stop=(i == t - 1))

                    steps = [("rect", kt) for kt in range(n_reg)]
                    steps += [("diag", j) for j in range(4)]
                    st = {}

                    def emit_scores(i, qt=qt, kt_t=kt_t, qs=qs, qi=qi, steps=steps, st=st):
                        kind, idx = steps[i]
                        psA = ps_sc.tile([128, QBLK], F32, tag="sA", name="psA")
                        psB = ps_sc.tile([128, QBLK], F32, tag="sB", name="psB")
                        pA = pt_pool.tile([128, QBLK], BF16, tag="pA", name="pA")
                        pB = pt_pool.tile([128, QBLK], BF16, tag="pB", name="pB")
                        if kind == "rect":
                            ks = slice(idx * 128, (idx + 1) * 128)
                            nc.tensor.matmul(
                                psA, lhsT=kt_t[0:64, ks], rhs=qt[0:64, qs],
                                start=True, stop=True,
                            )
                            nc.tensor.matmul(
                                psB, lhsT=kt_t[64:128, ks], rhs=qt[64:128, qs],
                                start=True, stop=True,
                            )
                            nc.scalar.activation(out=pA, in_=psA, func=EXP, scale=SCALE)
                            nc.scalar.activation(out=pB, in_=psB, func=EXP, scale=SCALE)
                            st[i] = (idx, 0, pA, pB)
                        else:
                            # diagonal tile: one N-restricted full-dst MM per
                            # half; keys 64:128 additionally need q >= q0+64,
                            # so that corner of p is zeroed before PV.
                            kt = 4 * qi + idx
                            q0 = 128 * idx
                            ks = slice(kt * 128, (kt + 1) * 128)
                            qsl0 = slice(qi * QBLK + q0, (qi + 1) * QBLK)
                            for ph, ps_s, p_s in ((0, psA, pA), (64, psB, pB)):
                                hd_sl = slice(ph, ph + 64)
                                nc.tensor.matmul(
                                    ps_s[:, q0:QBLK], lhsT=kt_t[hd_sl, ks],
                                    rhs=qt[hd_sl, qsl0], start=True, stop=True,
                                )
                                nc.scalar.activation(
                                    out=p_s[:, q0:QBLK], in_=ps_s[:, q0:QBLK],
                                    func=EXP, scale=SCALE,
                                )
                                nc.vector.tensor_copy(
                                    out=p_s[64:128, q0 : q0 + 64],
                                    in_=zeros_b[64:128, :],
                                )
                            st[i] = (kt, q0, pA, pB)

                    def emit_pv(i, vsl=vsl, st=st, fl=fl, at_A=at_A, at_B=at_B,
                                at_bA=at_bA, at_bB=at_bB):
                        kt, q0, pA, pB = st.pop(i)
                        nc.tensor.matmul(
                            at_bA[:, q0:QBLK], lhsT=vA[kt][:, vsl],
                            rhs=pA[:, q0:QBLK], **fl(at_A)
                        )
                        nc.tensor.matmul(
                            at_bB[:, q0:QBLK], lhsT=vB[kt][:, vsl],
                            rhs=pB[:, q0:QBLK], **fl(at_B)
                        )

                    emit_scores(0)
                    if norm_q:
                        norm_q.pop(0)()
                    if pair == 1 and out_q:
                        out_q.pop(0)()
                    for i in range(len(steps)):
                        if i + 1 < len(steps):
                            emit_scores(i + 1)
                        emit_pv(i)
                    norm_q.append(make_norm(pair, qs, at_bA, at_bB))
                out_q.append(make_outproj(qi))
            while norm_q:
                norm_q.pop(0)()
            while out_q:
                out_q.pop(0)()

    return nc


def _shard_inputs(x, w_qkv, w_proj):
    x = np.ascontiguousarray(np.asarray(x, dtype=np.float32))
    w_qkv = np.asarray(w_qkv, dtype=np.float32)
    w_proj = np.asarray(w_proj, dtype=np.float32)
    xT = [np.ascontiguousarray(x[b].T).reshape(8, 128, N) for b in range(B)]
    in_maps = []
    for c in range(NCORES):
        b, g = divmod(c, 4)
        r0 = 64 * HPC * g  # 256 * g
        wq = w_qkv[r0 : r0 + CSL, :]
        wk = w_qkv[C + r0 : C + r0 + CSL, :]
        wvs = w_qkv[2 * C + r0 : 2 * C + r0 + CSL, :]
        wqkT = np.ascontiguousarray(np.concatenate([wq, wk], axis=0).T)
        wvT = np.ascontiguousarray(wvs.T)
        wpT = np.ascontiguousarray(w_proj[:, r0 : r0 + CSL].T)
        in_maps.append(
            {
                "xT": xT[b],
                "wqkT": wqkT.reshape(8, 128, QKW),
                "wvT": wvT.reshape(8, 128, CSL),
                "wpT": wpT.reshape(2, 128, C),
            }
        )
    return in_maps


def run(x, w_qkv, w_proj, b_proj, trace=False, **spmd_kwargs):
    from concourse.bass_utils import run_bass_kernel_spmd

    in_maps = _shard_inputs(x, w_qkv, w_proj)
    nc = build_nc()
    nc.finalize()
    res = run_bass_kernel_spmd(
        nc, in_maps, core_ids=list(range(NCORES)), trace=trace, **spmd_kwargs
    )
    y = np.zeros((B, N, C), np.float32)
    for c in range(NCORES):
        y[c // 4] += np.asarray(res.results[c]["y"], dtype=np.float32)
    y += np.asarray(b_proj, dtype=np.float32)[None, None, :]
    return y, res


def kernel(x, w_qkv, w_proj, b_proj):
    y, _ = run(x, w_qkv, w_proj, b_proj, trace=False)
    return y


# revision 27
# speedup vs baseline: 2.4616x; 1.0761x over previous
"""Block-causal (block=64) MHA + qkv/out projections on 8 NeuronCores.

Sharding: 8 cores = 2 batches x 4 head-groups (4 heads each).
Per core: qkv projection for its heads, block-causal attention for 4 heads
(processed as 2 head-pairs packed across the 128 partitions), partial output
projection over its 256 channels. Host sums the 4 bf16 partials per batch
and adds the bias.

All matmuls run in bf16 at the full 1-cycle/row PE rate with cheap weight
loads (fp32 runs 4x slower; f32r loads stationaries ~4x slower and cannot
write partition-64-based PSUM dsts). x and the weights are pre-converted to
bf16 on the HOST, so they DMA straight into their SBUF tiles with no on-chip
casts and half the HBM traffic. PSUM accumulation stays fp32.

On-chip layout is feature-major (transposed): scores are computed transposed
(S^T[k, q] = k . q) so no on-chip transposes are needed anywhere.

Softmax denominators come for free from the PV matmul: each V tile carries
an appended all-ones column (plus zero padding to a full 128-wide stationary,
free since matmul time is per output row), so PV accumulator row 64 is the
running sum of exp. Denominator reciprocals are computed on the DVE with the
BITWISE_NOT-seed + 2 Newton-Raphson steps spelled out in STANDARD ops (the
fused custom-DVE op silently returns garbage on HW; ACT Ln/Exp thrashes
activation-table loads), then broadcast across 64 partitions with K=1
ones-matmuls. The B half's normalized rows are moved to attnT partitions
64:128 by an SBUF->SBUF DMA (engine ops cannot shift partitions; DMA can).

Emission is software-pipelined so the PE rarely waits: scores for step s+1
are emitted before the PV of step s, a pair's normalization is deferred
until after the next pair's first scores, and a query block's out-projection
is deferred into the next block's second pair. PSUM: 2 mm ring banks
(projections / out-proj / reciprocal-broadcasts), 1 bank per score half,
2x2 accumulator banks so consecutive pairs never contend.
"""

import numpy as np
import ml_dtypes

import concourse.bass as bass
import concourse.tile as tile
from concourse import bacc
from concourse import mybir

B, N, C = 2, 2048, 1024
H, HD = 16, 64
HPC = 4  # heads per core
CSL = HPC * HD  # 256 channel slice per core
QKW = 2 * CSL  # 512: q then k output channels
NCORES = 8
QBLK = 512
NQB = N // QBLK  # 4
NT = N // 128  # 16 seq tiles of 128
SCALE = HD**-0.5
F32 = mybir.dt.float32
BF16 = mybir.dt.bfloat16
EXP = mybir.ActivationFunctionType.Exp
COPY = mybir.ActivationFunctionType.Copy


def build_nc():
    nc = bacc.Bacc("TRN2", target_bir_lowering=False, debug=False, num_devices=NCORES)

    xT_d = nc.dram_tensor("xT", [8, 128, N], BF16, kind="ExternalInput")
    wqk_d = nc.dram_tensor("wqkT", [8, 128, QKW], BF16, kind="ExternalInput")
    wv_d = nc.dram_tensor("wvT", [8, 128, CSL], BF16, kind="ExternalInput")
    wp_d = nc.dram_tensor("wpT", [2, 128, C], BF16, kind="ExternalInput")
    y_d = nc.dram_tensor("y", [N, C], BF16, kind="ExternalOutput")

    with tile.TileContext(nc) as tc:
        with (
            tc.tile_pool(name="persist", bufs=1) as persist,
            tc.tile_pool(name="pt", bufs=2) as pt_pool,
            tc.tile_pool(name="rc", bufs=2) as rc_pool,
            tc.tile_pool(name="yout", bufs=3) as y_pool,
            tc.tile_pool(name="psmm", bufs=2, space="PSUM") as ps_mm,
            tc.tile_pool(name="pssc", bufs=1, space="PSUM") as ps_sc,
            tc.tile_pool(name="psacc", bufs=2, space="PSUM") as ps_acc,
        ):
            # ---- bf16 inputs DMA straight into their tiles ----
            wqk_bf = [persist.tile([128, QKW], BF16, tag=f"wqk{i}", name=f"wqk{i}") for i in range(8)]
            wv_bf = [persist.tile([128, CSL], BF16, tag=f"wv{i}", name=f"wv{i}") for i in range(8)]
            wp_bf = [persist.tile([128, C], BF16, tag=f"wp{i}", name=f"wp{i}") for i in range(2)]
            x_bf = [persist.tile([128, N], BF16, tag=f"xb{i}", name=f"xb{i}") for i in range(8)]
            for ct in range(8):
                nc.sync.dma_start(out=wqk_bf[ct], in_=wqk_d[ct])
            # x nb-major in 1024-col chunks so phase 1 starts early
            for half in range(2):
                for ct in range(8):
                    sl = slice(half * 1024, (half + 1) * 1024)
                    nc.sync.dma_start(out=x_bf[ct][:, sl], in_=xT_d[ct][:, sl])
            for ct in range(8):
                nc.sync.dma_start(out=wv_bf[ct], in_=wv_d[ct])
            for pr in range(2):
                nc.sync.dma_start(out=wp_bf[pr], in_=wp_d[pr])

            # constants: ones row for the K=1 broadcast matmuls (NR yields
            # MINUS the reciprocal, so the row is -1 to cancel the sign);
            # cz = [1 | 0*63] twice, the constant right half of every V tile.
            ones_f = persist.tile([128, 64], F32, tag="onesf")
            nc.vector.memset(ones_f, -1.0)
            onesel = persist.tile([128, 64], BF16, tag="sel")
            nc.vector.tensor_copy(out=onesel[64:65, :], in_=ones_f[64:65, :])
            cz_f = persist.tile([128, 128], F32, tag="czf")
            nc.vector.memset(cz_f, 0.0)
            nc.vector.memset(cz_f[:, 0:1], 1.0)
            nc.vector.memset(cz_f[:, 64:65], 1.0)
            cz3 = cz_f.rearrange("p (g c) -> p g c", c=64)

            # ---- phase 1: q/k projection, transposed bf16 outputs ----
            # qkT tiles: 0 = q heads(0,1), 1 = q heads(2,3), 2 = k(0,1), 3 = k(2,3)
            # within a tile: partitions 0:64 = even head dims, 64:128 = odd head.
            qkT = [persist.tile([128, N], BF16, tag=f"qk{t}", name=f"qk{t}") for t in range(4)]
            for nb in range(NQB):
                for dt_ in range(4):
                    ps = ps_mm.tile([128, QBLK], F32, tag="mm", name="ps_qk")
                    for ct in range(8):
                        nc.tensor.matmul(
                            ps,
                            lhsT=wqk_bf[ct][:, dt_ * 128 : (dt_ + 1) * 128],
                            rhs=x_bf[ct][:, nb * QBLK : (nb + 1) * QBLK],
                            start=(ct == 0),
                            stop=(ct == 7),
                        )
                    nc.vector.tensor_copy(
                        out=qkT[dt_][:, nb * QBLK : (nb + 1) * QBLK], in_=ps
                    )

            # ---- phase 2: v projection, with ones + zero-pad columns ----
            # vA/vB[nt] layout per pair p at cols 128p: [v(64) | ones | 0*63]
            # so the PV matmul writes a full 128-partition dst and its row 64
            # accumulates sum(exp); rows 65:128 accumulate zeros (free: cost
            # is per output row).
            vA = [persist.tile([128, 256], BF16, tag=f"vA{t}", name=f"vA{t}") for t in range(NT)]
            vB = [persist.tile([128, 256], BF16, tag=f"vB{t}", name=f"vB{t}") for t in range(NT)]
            for nt in range(NT):
                ps = ps_mm.tile([128, CSL], F32, tag="mm", name="ps_v")
                for ct in range(8):
                    nc.tensor.matmul(
                        ps,
                        lhsT=x_bf[ct][:, nt * 128 : (nt + 1) * 128],
                        rhs=wv_bf[ct],
                        start=(ct == 0),
                        stop=(ct == 7),
                    )
                ps3 = ps.rearrange("p (g c) -> p g c", c=128)  # [128, 2, 128]
                vA3 = vA[nt].rearrange("p (g c) -> p g c", c=128)
                vB3 = vB[nt].rearrange("p (g c) -> p g c", c=128)
                nc.vector.tensor_copy(out=vA3[:, :, 0:64], in_=ps3[:, :, 0:64])
                nc.vector.tensor_copy(out=vB3[:, :, 0:64], in_=ps3[:, :, 64:128])
                nc.vector.tensor_copy(out=vA3[:, :, 64:128], in_=cz3)
                nc.vector.tensor_copy(out=vB3[:, :, 64:128], in_=cz3)

            # ---- phase 3+4: attention (per 512-query block), then out-proj ----
            attnT = [persist.tile([128, N], BF16, tag=f"at{p}", name=f"at{p}") for p in range(2)]
            norm_q = []
            out_q = []

            def make_norm(pair, qs, at_bA, at_bB):
                def norm():
                    C0, C1, C2 = 0.23549792, 2.0017324, 2.0
                    I32 = mybir.dt.int32
                    XOR = mybir.AluOpType.bitwise_xor
                    MUL = mybir.AluOpType.mult
                    ADD = mybir.AluOpType.add
                    w1 = rc_pool.tile([128, 2 * QBLK], F32, tag="w1", name="w1")
                    w2 = rc_pool.tile([128, 2 * QBLK], F32, tag="w2", name="w2")
                    w3 = rc_pool.tile([128, 2 * QBLK], F32, tag="w3", name="w3")
                    rc_b = rc_pool.tile([128, 2 * QBLK], BF16, tag="rcb", name="rc_b")
                    r = slice(64, 65)
                    halves = (
                        (at_bA[r, :], slice(0, QBLK)),
                        (at_bB[r, :], slice(QBLK, 2 * QBLK)),
                    )
                    # z0 = ~bits(d) * (-c0): NR in the negated domain
                    # z_{k+1} = (d*z_k + Ck) * z_k keeps every step in the
                    # same (in0 op s) / tensor_tensor shapes; z2 = -1/d and
                    # the -1 broadcast row cancels the sign.
                    for d, cs in halves:
                        nc.vector.tensor_scalar(
                            out=w1[r, cs].bitcast(I32), in0=d.bitcast(I32),
                            scalar1=-1, scalar2=None, op0=XOR,
                        )
                    nc.vector.tensor_scalar_mul(w2[r, :], w1[r, :], C0)  # z0
                    for d, cs in halves:
                        nc.vector.tensor_mul(out=w1[r, cs], in0=d, in1=w2[r, cs])
                    nc.vector.tensor_scalar(
                        out=w3[r, :], in0=w1[r, :], scalar1=C1, scalar2=None, op0=ADD
                    )
                    nc.vector.tensor_mul(out=w1[r, :], in0=w2[r, :], in1=w3[r, :])
                    for d, cs in halves:  # z1 in w1
                        nc.vector.tensor_mul(out=w2[r, cs], in0=d, in1=w1[r, cs])
                    nc.vector.tensor_scalar(
                        out=w3[r, :], in0=w2[r, :], scalar1=C2, scalar2=None, op0=ADD
                    )
                    nc.vector.tensor_mul(out=rc_b[r, :], in0=w1[r, :], in1=w3[r, :])
                    # broadcast across 64 partitions with K=1 (-1)-matmuls
                    # (PSUM from the mm ring), staged through SBUF (tensor
                    # ops read at most one PSUM input).
                    bcA = ps_mm.tile([128, QBLK], F32, tag="mm", name="bcA")
                    bcB = ps_mm.tile([128, QBLK], F32, tag="mm", name="bcB")
                    nc.tensor.matmul(
                        bcA[0:64, :], lhsT=onesel[64:65, :],
                        rhs=rc_b[64:65, 0:QBLK], start=True, stop=True,
                    )
                    nc.tensor.matmul(
                        bcB[0:64, :], lhsT=onesel[64:65, :],
                        rhs=rc_b[64:65, QBLK : 2 * QBLK], start=True, stop=True,
                    )
                    bsA = rc_pool.tile([64, QBLK], F32, tag="bsA", name="bsA")
                    bsB = rc_pool.tile([64, QBLK], F32, tag="bsB", name="bsB")
                    nc.vector.tensor_copy(out=bsA, in_=bcA[0:64, :])
                    nc.vector.tensor_copy(out=bsB, in_=bcB[0:64, :])
                    nc.vector.tensor_mul(
                        out=attnT[pair][0:64, qs], in0=at_bA[0:64, :], in1=bsA
                    )
                    tmpB = rc_pool.tile([64, QBLK], BF16, tag="tmpB", name="tmpB")
                    nc.vector.tensor_mul(out=tmpB, in0=at_bB[0:64, :], in1=bsB)
                    # B's normalized rows move to partitions 64:128 (engine
                    # ops cannot shift partitions; SBUF->SBUF DMA can)
                    nc.sync.dma_start(out=attnT[pair][64:128, qs], in_=tmpB)
                return norm

            def make_outproj(qi):
                def op():
                    for nt in range(4 * qi, 4 * qi + 4):
                        ysb = y_pool.tile([128, C], BF16, tag="y", name="ysb")
                        for cb in range(2):
                            psy = ps_mm.tile([128, QBLK], F32, tag="mm", name="psy")
                            for pr in range(2):
                                nc.tensor.matmul(
                                    psy,
                                    lhsT=attnT[pr][:, nt * 128 : (nt + 1) * 128],
                                    rhs=wp_bf[pr][:, cb * QBLK : (cb + 1) * QBLK],
                                    start=(pr == 0),
                                    stop=(pr == 1),
                                )
                            # PSUM->SBUF on the Scalar engine (Copy shares
                            # the loaded activation table) to unload DVE
                            nc.scalar.activation(
                                out=ysb[:, cb * QBLK : (cb + 1) * QBLK], in_=psy,
                                func=COPY,
                            )
                        nc.sync.dma_start(
                            out=y_d[nt * 128 : (nt + 1) * 128, :], in_=ysb
                        )
                return op

            for qi in range(NQB):
                for pair in range(2):
                    qt = qkT[pair]
                    kt_t = qkT[2 + pair]
                    qs = slice(qi * QBLK, (qi + 1) * QBLK)
                    vsl = slice(pair * 128, (pair + 1) * 128)

                    # one PSUM bank per head half (2-deep ring so consecutive
                    # pairs never contend), softmax denominator in row 64.
                    at_bA = ps_acc.tile([128, QBLK], F32, tag="atA", name="at_bA")
                    at_bB = ps_acc.tile([128, QBLK], F32, tag="atB", name="at_bB")

                    n_reg = 4 * qi
                    total = n_reg + 4
                    at_A, at_B = [0], [0]

                    def fl(cnt, t=total):
                        i = cnt[0]
                        cnt[0] += 1
                        return dict(start=(i == 0), stop=(i == t - 1))

                    steps = [("rect", kt) for kt in range(n_reg)]
                    steps += [("diag", j) for j in range(4)]
                    st = {}

                    def emit_scores(i, qt=qt, kt_t=kt_t, qs=qs, qi=qi, steps=steps, st=st):
                        kind, idx = steps[i]
                        psA = ps_sc.tile([128, QBLK], F32, tag="sA", name="psA")
                        psB = ps_sc.tile([128, QBLK], F32, tag="sB", name="psB")
                        pA = pt_pool.tile([128, QBLK], BF16, tag="pA", name="pA")
                        pB = pt_pool.tile([128, QBLK], BF16, tag="pB", name="pB")
                        if kind == "rect":
                            ks = slice(idx * 128, (idx + 1) * 128)
                            nc.tensor.matmul(
                                psA, lhsT=kt_t[0:64, ks], rhs=qt[0:64, qs],
                                start=True, stop=True,
                            )
                            nc.tensor.matmul(
                                psB, lhsT=kt_t[64:128, ks], rhs=qt[64:128, qs],
                                start=True, stop=True,
                            )
                            nc.scalar.activation(out=pA, in_=psA, func=EXP, scale=SCALE)
                            nc.scalar.activation(out=pB, in_=psB, func=EXP, scale=SCALE)
                            st[i] = (idx, 0, pA, pB)
                        else:
                            # diagonal tile: one N-restricted full-dst MM per
                            # half; keys 64:128 additionally need q >= q0+64,
                            # so that corner of p is zeroed before PV.
                            kt = 4 * qi + idx
                            q0 = 128 * idx
                            ks = slice(kt * 128, (kt + 1) * 128)
                            qsl0 = slice(qi * QBLK + q0, (qi + 1) * QBLK)
                            for ph, ps_s, p_s in ((0, psA, pA), (64, psB, pB)):
                                hd_sl = slice(ph, ph + 64)
                                nc.tensor.matmul(
                                    ps_s[:, q0:QBLK], lhsT=kt_t[hd_sl, ks],
                                    rhs=qt[hd_sl, qsl0], start=True, stop=True,
                                )
                                nc.scalar.activation(
                                    out=p_s[:, q0:QBLK], in_=ps_s[:, q0:QBLK],
                                    func=EXP, scale=SCALE,
                                )
                                nc.gpsimd.memset(p_s[64:128, q0 : q0 + 64], 0.0)
                            st[i] = (kt, q0, pA, pB)

                    def emit_pv(i, vsl=vsl, st=st, fl=fl, at_A=at_A, at_B=at_B,
                                at_bA=at_bA, at_bB=at_bB):
                        kt, q0, pA, pB = st.pop(i)
                        nc.tensor.matmul(
                            at_bA[:, q0:QBLK], lhsT=vA[kt][:, vsl],
                            rhs=pA[:, q0:QBLK], **fl(at_A)
                        )
                        nc.tensor.matmul(
                            at_bB[:, q0:QBLK], lhsT=vB[kt][:, vsl],
                            rhs=pB[:, q0:QBLK], **fl(at_B)
                        )

                    emit_scores(0)
                    if norm_q:
                        norm_q.pop(0)()
                    if pair == 1 and out_q:
                        out_q.pop(0)()
                    for i in range(len(steps)):
                        if i + 1 < len(steps):
                            emit_scores(i + 1)
                        emit_pv(i)
                    norm_q.append(make_norm(pair, qs, at_bA, at_bB))
                out_q.append(make_outproj(qi))
            while norm_q:
                norm_q.pop(0)()
            while out_q:
                out_q.pop(0)()

    return nc


def _shard_inputs(x, w_qkv, w_proj):
    bf = ml_dtypes.bfloat16
    x = np.ascontiguousarray(np.asarray(x, dtype=np.float32))
    w_qkv = np.asarray(w_qkv, dtype=np.float32)
    w_proj = np.asarray(w_proj, dtype=np.float32)
    xT = [
        np.ascontiguousarray(x[b].T).astype(bf).reshape(8, 128, N) for b in range(B)
    ]
    in_maps = []
    for c in range(NCORES):
        b, g = divmod(c, 4)
        r0 = 64 * HPC * g  # 256 * g
        wq = w_qkv[r0 : r0 + CSL, :]
        wk = w_qkv[C + r0 : C + r0 + CSL, :]
        wvs = w_qkv[2 * C + r0 : 2 * C + r0 + CSL, :]
        wqkT = np.ascontiguousarray(np.concatenate([wq, wk], axis=0).T).astype(bf)
        wvT = np.ascontiguousarray(wvs.T).astype(bf)
        wpT = np.ascontiguousarray(w_proj[:, r0 : r0 + CSL].T).astype(bf)
        in_maps.append(
            {
                "xT": xT[b],
                "wqkT": wqkT.reshape(8, 128, QKW),
                "wvT": wvT.reshape(8, 128, CSL),
                "wpT": wpT.reshape(2, 128, C),
            }
        )
    return in_maps


def run(x, w_qkv, w_proj, b_proj, trace=False, **spmd_kwargs):
    from concourse.bass_utils import run_bass_kernel_spmd

    in_maps = _shard_inputs(x, w_qkv, w_proj)
    nc = build_nc()
    nc.finalize()
    res = run_bass_kernel_spmd(
        nc, in_maps, core_ids=list(range(NCORES)), trace=trace, **spmd_kwargs
    )
    y = np.zeros((B, N, C), np.float32)
    for c in range(NCORES):
        y[c // 4] += np.asarray(res.results[c]["y"], dtype=np.float32)
    y += np.asarray(b_proj, dtype=np.float32)[None, None, :]
    return y, res


def kernel(x, w_qkv, w_proj, b_proj):
    y, _ = run(x, w_qkv, w_proj, b_proj, trace=False)
    return y


# revision 29
# speedup vs baseline: 2.8593x; 1.1615x over previous
"""Block-causal (block=64) MHA + qkv/out projections on 8 NeuronCores.

Sharding: 8 cores = 2 batches x 4 head-groups (4 heads each).
Per core: qkv projection for its heads, block-causal attention for 4 heads
(processed as 2 head-pairs packed across the 128 partitions), partial output
projection over its 256 channels. Host sums the 4 bf16 partials per batch
and adds the bias.

All matmuls run in bf16 at the full 1-cycle/row PE rate with cheap weight
loads (fp32 runs 4x slower; f32r loads stationaries ~4x slower and cannot
write partition-64-based PSUM dsts). x and the weights are pre-converted to
bf16 on the HOST, so they DMA straight into their SBUF tiles with no on-chip
casts and half the HBM traffic. PSUM accumulation stays fp32.

On-chip layout is feature-major (transposed): scores are computed transposed
(S^T[k, q] = k . q) so no on-chip transposes are needed anywhere.

Softmax denominators come for free from the PV matmul: each V tile carries
an appended all-ones column (plus zero padding to a full 128-wide stationary,
free since matmul time is per output row), so PV accumulator row 64 is the
running sum of exp. Denominator reciprocals are computed on the DVE with the
BITWISE_NOT-seed + 2 Newton-Raphson steps spelled out in STANDARD ops (the
fused custom-DVE op silently returns garbage on HW; ACT Ln/Exp thrashes
activation-table loads), then broadcast across 64 partitions with K=1
ones-matmuls. The B half's normalized rows are moved to attnT partitions
64:128 by an SBUF->SBUF DMA (engine ops cannot shift partitions; DMA can).

Emission is software-pipelined so the PE rarely waits: scores for step s+1
are emitted before the PV of step s, a pair's normalization is deferred
until after the next pair's first scores, and a query block's out-projection
is deferred into the next block's second pair. PSUM: 2 mm ring banks
(projections / out-proj / reciprocal-broadcasts), 1 bank per score half,
2x2 accumulator banks so consecutive pairs never contend.
"""

import numpy as np
import ml_dtypes

import concourse.bass as bass
import concourse.tile as tile
from concourse import bacc
from concourse import mybir

B, N, C = 2, 2048, 1024
H, HD = 16, 64
HPC = 4  # heads per core
CSL = HPC * HD  # 256 channel slice per core
QKW = 2 * CSL  # 512: q then k output channels
NCORES = 8
QBLK = 512
NQB = N // QBLK  # 4
NT = N // 128  # 16 seq tiles of 128
SCALE = HD**-0.5
F32 = mybir.dt.float32
BF16 = mybir.dt.bfloat16
EXP = mybir.ActivationFunctionType.Exp
COPY = mybir.ActivationFunctionType.Copy


def build_nc():
    nc = bacc.Bacc("TRN2", target_bir_lowering=False, debug=False, num_devices=NCORES)

    xT_d = nc.dram_tensor("xT", [8, 128, N], BF16, kind="ExternalInput")
    wqk_d = nc.dram_tensor("wqkT", [8, 128, QKW], BF16, kind="ExternalInput")
    wv_d = nc.dram_tensor("wvT", [8, 128, CSL], BF16, kind="ExternalInput")
    wp_d = nc.dram_tensor("wpT", [2, 128, C], BF16, kind="ExternalInput")
    y_d = nc.dram_tensor("y", [N, C], BF16, kind="ExternalOutput")

    with tile.TileContext(nc) as tc:
        with (
            tc.tile_pool(name="persist", bufs=1) as persist,
            tc.tile_pool(name="pt", bufs=2) as pt_pool,
            tc.tile_pool(name="rc", bufs=2) as rc_pool,
            tc.tile_pool(name="yout", bufs=3) as y_pool,
            tc.tile_pool(name="psmm", bufs=2, space="PSUM") as ps_mm,
            tc.tile_pool(name="pssc", bufs=1, space="PSUM") as ps_sc,
            tc.tile_pool(name="psacc", bufs=2, space="PSUM") as ps_acc,
        ):
            # ---- bf16 inputs DMA straight into their tiles ----
            wqk_bf = [persist.tile([128, QKW], BF16, tag=f"wqk{i}", name=f"wqk{i}") for i in range(8)]
            wv_bf = [persist.tile([128, CSL], BF16, tag=f"wv{i}", name=f"wv{i}") for i in range(8)]
            wp_bf = [persist.tile([128, C], BF16, tag=f"wp{i}", name=f"wp{i}") for i in range(2)]
            x_bf = [persist.tile([128, N], BF16, tag=f"xb{i}", name=f"xb{i}") for i in range(8)]
            for ct in range(8):
                nc.sync.dma_start(out=wqk_bf[ct], in_=wqk_d[ct])
            # x nb-major in 1024-col chunks so phase 1 starts early
            for half in range(2):
                for ct in range(8):
                    sl = slice(half * 1024, (half + 1) * 1024)
                    nc.sync.dma_start(out=x_bf[ct][:, sl], in_=xT_d[ct][:, sl])
            for ct in range(8):
                nc.sync.dma_start(out=wv_bf[ct], in_=wv_d[ct])
            for pr in range(2):
                nc.sync.dma_start(out=wp_bf[pr], in_=wp_d[pr])

            # constants: -1 row for the K=1 broadcast matmuls (NR yields
            # MINUS the reciprocal; -1 cancels the sign); cz = [1 | 0*63]
            # twice, the constant right half of every V tile.
            ones_f = persist.tile([128, 64], F32, tag="onesf")
            nc.vector.memset(ones_f, -1.0)
            onesel = persist.tile([128, 64], BF16, tag="sel")
            nc.vector.tensor_copy(out=onesel[64:65, :], in_=ones_f[64:65, :])
            cz_f = persist.tile([128, 128], F32, tag="czf")
            nc.vector.memset(cz_f, 0.0)
            nc.vector.memset(cz_f[:, 0:1], 1.0)
            nc.vector.memset(cz_f[:, 64:65], 1.0)
            cz3 = cz_f.rearrange("p (g c) -> p g c", c=64)

            # ---- phases 1+2 (projections), emitted PER BLOCK inside the
            # attention loop so the Scalar engine's exp work overlaps the
            # projection matmuls instead of idling through them.
            # qkT tiles: 0 = q heads(0,1), 1 = q heads(2,3), 2 = k(0,1), 3 = k(2,3)
            # within a tile: partitions 0:64 = even head dims, 64:128 = odd head.
            qkT = [persist.tile([128, N], BF16, tag=f"qk{t}", name=f"qk{t}") for t in range(4)]
            # vA/vB[nt] layout per pair p at cols 128p: [v(64) | ones | 0*63]
            # so the PV matmul writes a full 128-partition dst and its row 64
            # accumulates sum(exp); rows 65:128 accumulate zeros (free: cost
            # is per output row).
            vA = [persist.tile([128, 256], BF16, tag=f"vA{t}", name=f"vA{t}") for t in range(NT)]
            vB = [persist.tile([128, 256], BF16, tag=f"vB{t}", name=f"vB{t}") for t in range(NT)]

            def emit_ph1(nb):
                for dt_ in range(4):
                    ps = ps_mm.tile([128, QBLK], F32, tag="mm", name="ps_qk")
                    for ct in range(8):
                        nc.tensor.matmul(
                            ps,
                            lhsT=wqk_bf[ct][:, dt_ * 128 : (dt_ + 1) * 128],
                            rhs=x_bf[ct][:, nb * QBLK : (nb + 1) * QBLK],
                            start=(ct == 0),
                            stop=(ct == 7),
                        )
                    nc.vector.tensor_copy(
                        out=qkT[dt_][:, nb * QBLK : (nb + 1) * QBLK], in_=ps
                    )

            def emit_ph2(nt):
                ps = ps_mm.tile([128, CSL], F32, tag="mm", name="ps_v")
                for ct in range(8):
                    nc.tensor.matmul(
                        ps,
                        lhsT=x_bf[ct][:, nt * 128 : (nt + 1) * 128],
                        rhs=wv_bf[ct],
                        start=(ct == 0),
                        stop=(ct == 7),
                    )
                ps3 = ps.rearrange("p (g c) -> p g c", c=128)  # [128, 2, 128]
                vA3 = vA[nt].rearrange("p (g c) -> p g c", c=128)
                vB3 = vB[nt].rearrange("p (g c) -> p g c", c=128)
                nc.vector.tensor_copy(out=vA3[:, :, 0:64], in_=ps3[:, :, 0:64])
                nc.vector.tensor_copy(out=vB3[:, :, 0:64], in_=ps3[:, :, 64:128])
                nc.vector.tensor_copy(out=vA3[:, :, 64:128], in_=cz3)
                nc.vector.tensor_copy(out=vB3[:, :, 64:128], in_=cz3)

            # ---- phase 3+4: attention (per 512-query block), then out-proj ----
            attnT = [persist.tile([128, N], BF16, tag=f"at{p}", name=f"at{p}") for p in range(2)]
            norm_q = []
            out_q = []

            def make_norm(pair, qs, at_bA, at_bB):
                def norm():
                    C0, C1, C2 = 0.23549792, 2.0017324, 2.0
                    I32 = mybir.dt.int32
                    XOR = mybir.AluOpType.bitwise_xor
                    MUL = mybir.AluOpType.mult
                    ADD = mybir.AluOpType.add
                    w1 = rc_pool.tile([128, 2 * QBLK], F32, tag="w1", name="w1")
                    w2 = rc_pool.tile([128, 2 * QBLK], F32, tag="w2", name="w2")
                    w3 = rc_pool.tile([128, 2 * QBLK], F32, tag="w3", name="w3")
                    rc_b = rc_pool.tile([128, 2 * QBLK], BF16, tag="rcb", name="rc_b")
                    r = slice(64, 65)
                    halves = (
                        (at_bA[r, :], slice(0, QBLK)),
                        (at_bB[r, :], slice(QBLK, 2 * QBLK)),
                    )
                    # z0 = ~bits(d) * (-c0): NR in the negated domain
                    # z_{k+1} = (d*z_k + Ck) * z_k keeps every step in the
                    # same (in0 op s) / tensor_tensor shapes; z2 = -1/d and
                    # the -1 broadcast row cancels the sign.
                    for d, cs in halves:
                        nc.vector.tensor_scalar(
                            out=w1[r, cs].bitcast(I32), in0=d.bitcast(I32),
                            scalar1=-1, scalar2=None, op0=XOR,
                        )
                    nc.vector.tensor_scalar_mul(w2[r, :], w1[r, :], C0)  # z0
                    for d, cs in halves:
                        nc.vector.tensor_mul(out=w1[r, cs], in0=d, in1=w2[r, cs])
                    nc.vector.tensor_scalar(
                        out=w3[r, :], in0=w1[r, :], scalar1=C1, scalar2=None, op0=ADD
                    )
                    nc.vector.tensor_mul(out=w1[r, :], in0=w2[r, :], in1=w3[r, :])
                    for d, cs in halves:  # z1 in w1
                        nc.vector.tensor_mul(out=w2[r, cs], in0=d, in1=w1[r, cs])
                    nc.vector.tensor_scalar(
                        out=w3[r, :], in0=w2[r, :], scalar1=C2, scalar2=None, op0=ADD
                    )
                    nc.vector.tensor_mul(out=rc_b[r, :], in0=w1[r, :], in1=w3[r, :])
                    # broadcast across 64 partitions with K=1 (-1)-matmuls
                    # (z2 = -1/d, the -1 row cancels the sign; PSUM from the
                    # mm ring), staged through SBUF (tensor ops read at most
                    # one PSUM input; stride-0 DMA broadcast is rejected).
                    bcA = ps_mm.tile([128, QBLK], F32, tag="mm", name="bcA")
                    bcB = ps_mm.tile([128, QBLK], F32, tag="mm", name="bcB")
                    nc.tensor.matmul(
                        bcA[0:64, :], lhsT=onesel[64:65, :],
                        rhs=rc_b[64:65, 0:QBLK], start=True, stop=True,
                    )
                    nc.tensor.matmul(
                        bcB[0:64, :], lhsT=onesel[64:65, :],
                        rhs=rc_b[64:65, QBLK : 2 * QBLK], start=True, stop=True,
                    )
                    bsA = rc_pool.tile([64, QBLK], F32, tag="bsA", name="bsA")
                    bsB = rc_pool.tile([64, QBLK], F32, tag="bsB", name="bsB")
                    nc.vector.tensor_copy(out=bsA, in_=bcA[0:64, :])
                    nc.vector.tensor_copy(out=bsB, in_=bcB[0:64, :])
                    nc.vector.tensor_mul(
                        out=attnT[pair][0:64, qs], in0=at_bA[0:64, :], in1=bsA
                    )
                    tmpB = rc_pool.tile([64, QBLK], BF16, tag="tmpB", name="tmpB")
                    nc.vector.tensor_mul(out=tmpB, in0=at_bB[0:64, :], in1=bsB)
                    # B's normalized rows move to partitions 64:128 (engine
                    # ops cannot shift partitions; SBUF->SBUF DMA can)
                    nc.sync.dma_start(out=attnT[pair][64:128, qs], in_=tmpB)
                return norm

            def make_outproj(qi):
                def op():
                    for nt in range(4 * qi, 4 * qi + 4):
                        ysb = y_pool.tile([128, C], BF16, tag="y", name="ysb")
                        for cb in range(2):
                            psy = ps_mm.tile([128, QBLK], F32, tag="mm", name="psy")
                            for pr in range(2):
                                nc.tensor.matmul(
                                    psy,
                                    lhsT=attnT[pr][:, nt * 128 : (nt + 1) * 128],
                                    rhs=wp_bf[pr][:, cb * QBLK : (cb + 1) * QBLK],
                                    start=(pr == 0),
                                    stop=(pr == 1),
                                )
                            # PSUM->SBUF on the Scalar engine (Copy shares
                            # the loaded activation table) to unload DVE
                            nc.scalar.activation(
                                out=ysb[:, cb * QBLK : (cb + 1) * QBLK], in_=psy,
                                func=COPY,
                            )
                        nc.sync.dma_start(
                            out=y_d[nt * 128 : (nt + 1) * 128, :], in_=ysb
                        )
                return op

            for qi in range(NQB):
                emit_ph1(qi)
                for nt in range(4 * qi, 4 * qi + 4):
                    emit_ph2(nt)
                for pair in range(2):
                    qt = qkT[pair]
                    kt_t = qkT[2 + pair]
                    qs = slice(qi * QBLK, (qi + 1) * QBLK)
                    vsl = slice(pair * 128, (pair + 1) * 128)

                    # one PSUM bank per head half (2-deep ring so consecutive
                    # pairs never contend), softmax denominator in row 64.
                    at_bA = ps_acc.tile([128, QBLK], F32, tag="atA", name="at_bA")
                    at_bB = ps_acc.tile([128, QBLK], F32, tag="atB", name="at_bB")

                    n_reg = 4 * qi
                    total = n_reg + 4
                    at_A, at_B = [0], [0]

                    def fl(cnt, t=total):
                        i = cnt[0]
                        cnt[0] += 1
                        return dict(start=(i == 0), stop=(i == t - 1))

                    steps = [("rect", kt) for kt in range(n_reg)]
                    steps += [("diag", j) for j in range(4)]
                    st = {}

                    def emit_scores(i, qt=qt, kt_t=kt_t, qs=qs, qi=qi, steps=steps, st=st):
                        kind, idx = steps[i]
                        psA = ps_sc.tile([128, QBLK], F32, tag="sA", name="psA")
                        psB = ps_sc.tile([128, QBLK], F32, tag="sB", name="psB")
                        pA = pt_pool.tile([128, QBLK], BF16, tag="pA", name="pA")
                        pB = pt_pool.tile([128, QBLK], BF16, tag="pB", name="pB")
                        if kind == "rect":
                            ks = slice(idx * 128, (idx + 1) * 128)
                            nc.tensor.matmul(
                                psA, lhsT=kt_t[0:64, ks], rhs=qt[0:64, qs],
                                start=True, stop=True,
                            )
                            nc.tensor.matmul(
                                psB, lhsT=kt_t[64:128, ks], rhs=qt[64:128, qs],
                                start=True, stop=True,
                            )
                            nc.scalar.activation(out=pA, in_=psA, func=EXP, scale=SCALE)
                            nc.scalar.activation(out=pB, in_=psB, func=EXP, scale=SCALE)
                            st[i] = (idx, 0, pA, pB)
                        else:
                            # diagonal tile: one N-restricted full-dst MM per
                            # half; keys 64:128 additionally need q >= q0+64,
                            # so that corner of p is zeroed before PV.
                            kt = 4 * qi + idx
                            q0 = 128 * idx
                            ks = slice(kt * 128, (kt + 1) * 128)
                            qsl0 = slice(qi * QBLK + q0, (qi + 1) * QBLK)
                            for ph, ps_s, p_s in ((0, psA, pA), (64, psB, pB)):
                                hd_sl = slice(ph, ph + 64)
                                nc.tensor.matmul(
                                    ps_s[:, q0:QBLK], lhsT=kt_t[hd_sl, ks],
                                    rhs=qt[hd_sl, qsl0], start=True, stop=True,
                                )
                                nc.scalar.activation(
                                    out=p_s[:, q0:QBLK], in_=ps_s[:, q0:QBLK],
                                    func=EXP, scale=SCALE,
                                )
                                nc.gpsimd.memset(p_s[64:128, q0 : q0 + 64], 0.0)
                            st[i] = (kt, q0, pA, pB)

                    def emit_pv(i, vsl=vsl, st=st, fl=fl, at_A=at_A, at_B=at_B,
                                at_bA=at_bA, at_bB=at_bB):
                        kt, q0, pA, pB = st.pop(i)
                        nc.tensor.matmul(
                            at_bA[:, q0:QBLK], lhsT=vA[kt][:, vsl],
                            rhs=pA[:, q0:QBLK], **fl(at_A)
                        )
                        nc.tensor.matmul(
                            at_bB[:, q0:QBLK], lhsT=vB[kt][:, vsl],
                            rhs=pB[:, q0:QBLK], **fl(at_B)
                        )

                    emit_scores(0)
                    if norm_q:
                        norm_q.pop(0)()
                    if pair == 1 and out_q:
                        out_q.pop(0)()
                    for i in range(len(steps)):
                        if i + 1 < len(steps):
                            emit_scores(i + 1)
                        emit_pv(i)
                    norm_q.append(make_norm(pair, qs, at_bA, at_bB))
                out_q.append(make_outproj(qi))
            while norm_q:
                norm_q.pop(0)()
            while out_q:
                out_q.pop(0)()

    return nc


def _shard_inputs(x, w_qkv, w_proj):
    bf = ml_dtypes.bfloat16
    x = np.ascontiguousarray(np.asarray(x, dtype=np.float32))
    w_qkv = np.asarray(w_qkv, dtype=np.float32)
    w_proj = np.asarray(w_proj, dtype=np.float32)
    xT = [
        np.ascontiguousarray(x[b].T).astype(bf).reshape(8, 128, N) for b in range(B)
    ]
    in_maps = []
    for c in range(NCORES):
        b, g = divmod(c, 4)
        r0 = 64 * HPC * g  # 256 * g
        wq = w_qkv[r0 : r0 + CSL, :]
        wk = w_qkv[C + r0 : C + r0 + CSL, :]
        wvs = w_qkv[2 * C + r0 : 2 * C + r0 + CSL, :]
        wqkT = np.ascontiguousarray(np.concatenate([wq, wk], axis=0).T).astype(bf)
        wvT = np.ascontiguousarray(wvs.T).astype(bf)
        wpT = np.ascontiguousarray(w_proj[:, r0 : r0 + CSL].T).astype(bf)
        in_maps.append(
            {
                "xT": xT[b],
                "wqkT": wqkT.reshape(8, 128, QKW),
                "wvT": wvT.reshape(8, 128, CSL),
                "wpT": wpT.reshape(2, 128, C),
            }
        )
    return in_maps


def run(x, w_qkv, w_proj, b_proj, trace=False, **spmd_kwargs):
    from concourse.bass_utils import run_bass_kernel_spmd

    in_maps = _shard_inputs(x, w_qkv, w_proj)
    nc = build_nc()
    nc.finalize()
    res = run_bass_kernel_spmd(
        nc, in_maps, core_ids=list(range(NCORES)), trace=trace, **spmd_kwargs
    )
    y = np.zeros((B, N, C), np.float32)
    for c in range(NCORES):
        y[c // 4] += np.asarray(res.results[c]["y"], dtype=np.float32)
    y += np.asarray(b_proj, dtype=np.float32)[None, None, :]
    return y, res


def kernel(x, w_qkv, w_proj, b_proj):
    y, _ = run(x, w_qkv, w_proj, b_proj, trace=False)
    return y
